# revision 1
# baseline (speedup 1.0000x reference)
"""Trainium2 Bass kernel for nn_Aggre_social (GNN message passing, social rec).

Strategy: data-parallel over the B=512 query users across 8 NeuronCores
(64 queries/core).  Per core we compute item_final() for 64 self uids +
64*32 = 2048 neighbor uids = 2112 uids (padded to 17 tiles of 128), each
with a 64-item history.

The dominant dispatch cost is host->device staging through the axon
tunnel (~40MB/s), so inputs are minimized:
  - the folded item table itw1 = item_w @ i_ln1_w[:64] is staged row-
    SHARDED (6250 rows/core, 800KB) and AllGather'd on device;
  - per-(uid,item) item ids ship as uint16 rows (ihist[uids]), ratings
    and masks as uint8 rows; item embedding rows are gathered on device
    with indirect DMA (one [128,1]-index instruction per item column);
  - the 5-row rating table rtw1 = rating_w @ i_ln1_w[64:] + b1 is added
    via 5 select-multiply-add vector ops per tile (no gather);
  - user rows are host-gathered (2176 rows/core, bf16).
Compute is done mostly in bf16 with a feature-major ("transposed")
layout for the attention MLPs and a row-major layout for softmax /
weighted sums.
"""

import sys

sys.path.insert(0, "/opt/trn_rl_repo")

import numpy as np
import ml_dtypes
from contextlib import ExitStack

import concourse.bass as bass
import concourse.bacc as bacc
import concourse.tile as tile
from concourse import mybir
from concourse.bass import IndirectOffsetOnAxis, AP
from concourse.bass_utils import run_bass_kernel_spmd

BF16 = mybir.dt.bfloat16
F32 = mybir.dt.float32
I32 = mybir.dt.int32
U16 = mybir.dt.uint16
U8 = mybir.dt.uint8
F8E4 = mybir.dt.float8e4
AF = mybir.ActivationFunctionType
ALU = mybir.AluOpType
AXL = mybir.AxisListType

NCORES = 8
B = 512
BC = B // NCORES          # 64 queries per core
LS = 32
LI = 64
D = 64
NU = 50000
SHROWS = NU // NCORES     # 6250 fp8 itw1 rows staged per core
WBROWS = 240              # wb weight-blob bf16 rows appended per shard
# blob region offsets, in 64-elem bf16 rows (128B each)
R_TAB = 0                 # fp8 item table shard: 6250 x 64B = 3125 rows
R_WB = 3125               # wb blob chunk: 240 rows
AGROWS = R_WB + WBROWS    # 3365 rows participate in the AllGather
R_USER = AGROWS           # fp8 user rows: 2176 x 64B = 1088 rows
R_IHG = R_USER + 1088     # u16 AG-mapped item ids: 2176 rows
R_AUX = R_IHG + 2176      # 4-bit packed 2*rating+mask: 2176x32B = 544 rows
R_ROWS = R_AUX + 544      # bias row-vectors: 7 rows
R_RT = R_ROWS + 7         # folded rating rows: 5 rows
R_SMB = R_RT + 5          # social mask bias f32 [64,32]: 64 rows
R_FB = R_SMB + 64         # per-partition f32 bias/scale cols [128,4]: 16
RTOT = R_FB + 16          # 7265 rows = 930KB per core
TABROWS = NCORES * AGROWS  # 26920 bf16 rows in the AllGather'd table
FP8SH = 2 * AGROWS        # 6730 fp8 table-row slots per AG block
NT = 17                   # uid tiles per core (128 uids each)
NUID = NT * 128           # 2176 (2112 real + 64 pad)
NT2 = 18                  # padded to even for pair-blocking in phase 2
HIFROWS = NT2 * 128       # 2304 rows in HIF scratch

nbf = ml_dtypes.bfloat16

_CACHE = {}


# ---------------------------------------------------------------------------
# run_bass_via_pjrt rebuilds its jit closure (and so retraces + relowers the
# whole program) on every call, which costs ~0.5s/dispatch of pure client-side
# work.  Memoize the jitted sharded callable per Bass program: semantics are
# identical (inputs are still concatenated, staged to the devices, executed
# and fetched on every call) -- only the jax trace/lower step is reused.
# run_bass_kernel_spmd remains the dispatch entry point.
# ---------------------------------------------------------------------------
_PJRT_JIT_CACHE = {}


def _install_pjrt_cache():
    from concourse import bass2jax as b2j
    import jax
    from jax.sharding import Mesh, PartitionSpec
    from jax.experimental.shard_map import shard_map

    orig = b2j.run_bass_via_pjrt
    if getattr(orig, "_aggre_cached", False):
        return

    def cached_run(nc, in_maps, n_cores):
        key = (id(nc), n_cores)
        if key not in _PJRT_JIT_CACHE:
            if nc.dbg_addr is not None or n_cores == 1:
                return orig(nc, in_maps, n_cores)  # uncommon paths: passthru
            b2j.install_neuronx_cc_hook()
            partition_name = (nc.partition_id_tensor.name
                              if nc.partition_id_tensor else None)
            in_names, out_names, out_avals, zero_shapes = [], [], [], []
            for alloc in nc.m.functions[0].allocations:
                if not isinstance(alloc, mybir.MemoryLocationSet):
                    continue
                name = alloc.memorylocations[0].name
                if alloc.kind == "ExternalInput":
                    if name != partition_name:
                        in_names.append(name)
                elif alloc.kind == "ExternalOutput":
                    out_names.append(name)
                    shape = tuple(alloc.tensor_shape)
                    dtype = mybir.dt.np(alloc.dtype)
                    out_avals.append(jax.core.ShapedArray(shape, dtype))
                    zero_shapes.append((shape, dtype))
            n_params = len(in_names)
            all_names = list(in_names) + out_names
            if partition_name is not None:
                all_names.append(partition_name)
            donate = tuple(range(n_params, n_params + len(out_names)))

            def _body(*args):
                operands = list(args)
                if partition_name is not None:
                    operands.append(b2j.partition_id_tensor())
                outs = b2j._bass_exec_p.bind(
                    *operands, out_avals=tuple(out_avals),
                    in_names=tuple(all_names), out_names=tuple(out_names),
                    lowering_input_output_aliases=(),
                    sim_require_finite=True, sim_require_nnan=True, nc=nc)
                return tuple(outs)

            mesh = Mesh(np.asarray(jax.devices()[:n_cores]), ("core",))
            sharded = jax.jit(
                shard_map(_body, mesh=mesh,
                          in_specs=(PartitionSpec("core"),)
                          * (n_params + len(out_names)),
                          out_specs=(PartitionSpec("core"),)
                          * len(out_names), check_rep=False),
                donate_argnums=donate, keep_unused=True)
            _PJRT_JIT_CACHE[key] = (nc, sharded, in_names, out_names,
                                    out_avals, zero_shapes)
        (_, sharded, in_names, out_names, out_avals,
         zero_shapes) = _PJRT_JIT_CACHE[key]
        n_params = len(in_names)
        concat_in = [
            np.concatenate([np.asarray(in_maps[c][name])
                            for c in range(n_cores)], axis=0)
            for name in in_names]
        # donate the previous call's device-resident output buffers (the
        # kernel overwrites every output element); zeros on the first call
        prev = _PJRT_JIT_CACHE.get(("prev_out", key))
        if prev is None:
            prev = [np.zeros((n_cores * s[0], *s[1:]), dt)
                    for s, dt in zero_shapes]
        out_arrs = sharded(*concat_in, *prev)
        _PJRT_JIT_CACHE[("prev_out", key)] = list(out_arrs)
        if getattr(nc, "_replicated_out", False):
            # every core holds the full result: fetch one shard only
            res0 = {name: np.asarray(out_arrs[i].addressable_shards[0].data)
                    for i, name in enumerate(out_names)}
            return [res0 for _ in range(n_cores)]
        return [
            {name: np.asarray(out_arrs[i]).reshape(
                n_cores, *out_avals[i].shape)[c]
             for i, name in enumerate(out_names)}
            for c in range(n_cores)]

    cached_run._aggre_cached = True
    b2j.run_bass_via_pjrt = cached_run


_install_pjrt_cache()


def _bcast_mid(ap, rep):
    """[P, n] AP -> [P, rep, n] AP with stride-0 middle dim."""
    assert len(ap.ap) == 2
    return AP(ap.tensor, ap.offset, [ap.ap[0], [0, rep], ap.ap[1]])


def _bcast_inner(ap, rep):
    """[P, n] AP -> [P, n, rep] AP with stride-0 inner dim."""
    assert len(ap.ap) == 2
    return AP(ap.tensor, ap.offset, [ap.ap[0], ap.ap[1], [0, rep]])


def _stride2(ap, phase, n):
    """[P, 2n] AP -> [P, n] AP over elements phase, phase+2, ..."""
    assert len(ap.ap) == 2
    return AP(ap.tensor, ap.offset + phase, [ap.ap[0], [2, n]])


def build_program(full_tab=False):
    """full_tab=True replaces the table AllGather with a full-table input
    (single-core CoreSim testing)."""
    nc = bacc.Bacc("TRN2", target_bir_lowering=False, debug=False,
                   num_devices=NCORES)

    # ---- DRAM I/O ------------------------------------------------------
    def din(name, shape, dt):
        return nc.dram_tensor(name, shape, dt, kind="ExternalInput").ap()

    t_big = din("big", [RTOT, D], BF16)      # everything (see R_* layout)
    if full_tab:
        t_tabin = din("tabf", [TABROWS, D], BF16)  # prebuilt AG result
    t_out = nc.dram_tensor("out", [B, D], F32, kind="ExternalOutput").ap()

    W = {}  # weight blob column slots
    for i, name in enumerate(["A1top", "A1bot", "A2", "Wl2", "Wl3top",
                              "Wl3bot", "sA1top", "sA1bot", "sA2", "sWl1",
                              "sWl2top", "sWl2bot", "sWl3"]):
        W[name] = i * 64
    IDB_OFF = 13 * 64
    RW = {n: i * 64 for i, n in enumerate(
        ["b2a", "b2l", "b3l", "b2s", "b1ls", "b2ls", "b3ls"])}
    FB = {"b1a2": 0, "a3pk": 1, "b1s2": 2, "a3s2": 3}

    with tile.TileContext(nc) as tc, ExitStack() as ctx:
        const_p = ctx.enter_context(tc.tile_pool(name="const", bufs=1))
        small_p = ctx.enter_context(tc.tile_pool(name="small", bufs=3))
        big_p = ctx.enter_context(tc.tile_pool(name="big", bufs=2))
        chunk_p = ctx.enter_context(tc.tile_pool(name="chunk", bufs=4))
        pers_p = ctx.enter_context(tc.tile_pool(name="pers", bufs=1))
        dram_p = ctx.enter_context(tc.tile_pool(name="dram", bufs=1,
                                                space="DRAM"))
        ps_tr = ctx.enter_context(tc.tile_pool(name="ps_tr", bufs=2,
                                               space="PSUM"))
        ps_mm = ctx.enter_context(tc.tile_pool(name="ps_mm", bufs=2,
                                               space="PSUM"))
        ps_bk = ctx.enter_context(tc.tile_pool(name="ps_bk", bufs=2,
                                               space="PSUM"))
        ps_sm = ctx.enter_context(tc.tile_pool(name="ps_sm", bufs=1,
                                               space="PSUM"))

        # ---- item table: shard in, AllGather to full table -------------
        if full_tab:
            tab = dram_p.tile([TABROWS, D], BF16, tag="tab")
            nc.gpsimd.dma_start(tab[:], t_tabin[:])
        else:
            shard_b = dram_p.tile([AGROWS, D], BF16, tag="shard")
            tab = dram_p.tile([TABROWS, D], BF16, tag="tab",
                              addr_space="Shared")
            nc.gpsimd.dma_start(shard_b[:], t_big[R_TAB:AGROWS, :])
            nc.gpsimd.collective_compute(
                "AllGather", ALU.bypass,
                replica_groups=[list(range(NCORES))],
                ins=[shard_b.opt()], outs=[tab.opt()])
        # fp8 view of the gathered table, one 64-byte row per item slot
        tab8 = tab[:].bitcast(U8).rearrange("r (a g) -> (r a) g", g=64)

        # ---- constants into SBUF --------------------------------------
        # wb rides in the AG'd shard: rank c's appendix rows hold wb
        # partitions [16c, 16c+16) (15 col-groups of 64 each)
        wb = const_p.tile([128, 13 * 64 + 128], BF16)
        for c in range(NCORES):
            nc.sync.dma_start(
                wb[16 * c:16 * (c + 1), :],
                tab[c * AGROWS + R_WB:c * AGROWS + R_WB + WBROWS, :])
        rows = const_p.tile([1, 7 * 64], BF16)
        nc.sync.dma_start(rows[:], t_big[R_ROWS:R_ROWS + 7, :])
        fb = const_p.tile([128, 4], F32)
        nc.sync.dma_start(fb[:], t_big[R_FB:R_FB + 16, :].bitcast(F32))
        smb = const_p.tile([BC, LS], F32)
        nc.sync.dma_start(smb[:], t_big[R_SMB:R_SMB + 64, :].bitcast(F32))
        ones = const_p.tile([1, 512], BF16)
        nc.vector.memset(ones[:], 1.0)
        idb = wb[:, IDB_OFF:IDB_OFF + 128]      # bf16 identity
        idf = const_p.tile([128, 128], F32)     # f32 identity (0/1 exact)
        nc.vector.tensor_copy(idf[:], idb)

        # rating rows -> broadcast to all 128 partitions: rep5[p, k*64+f]
        rtrow = const_p.tile([1, 5 * D], BF16)
        nc.sync.dma_start(rtrow[:], t_big[R_RT:R_RT + 5, :])
        ps_r5 = ps_mm.tile([128, 5 * D], F32, tag="mm")
        nc.tensor.matmul(ps_r5[:], lhsT=ones[:, 0:128], rhs=rtrow[:],
                         start=True, stop=True)
        rep5 = const_p.tile([128, 5 * D], BF16)
        nc.vector.tensor_copy(rep5[:], ps_r5[:])

        def wsl(name):
            return wb[:, W[name]:W[name] + 64]

        def rowv(name):
            return rows[:, RW[name]:RW[name] + 64]

        def fbc(name):
            return fb[:, FB[name]:FB[name] + 1]

        HI_all = pers_p.tile([128, NT2 * 64], BF16)
        U_all = pers_p.tile([128, NT2 * 64], BF16)
        nc.vector.memset(HI_all[:, NT * 64:], 0.0)
        nc.vector.memset(U_all[:, NT * 64:], 0.0)
        hif = dram_p.tile([HIFROWS, D], BF16, tag="hif")
        assert hif[:].offset == 0

        # ================= PHASE 1: item_final per uid-tile =============
        for t in range(NT):
            # user rows arrive fp8; widen to bf16 once
            UR8 = small_p.tile([128, D], U8, tag="UR8")
            nc.sync.dma_start(
                UR8[:],
                t_big[R_USER + t * 64:R_USER + (t + 1) * 64, :].bitcast(U8))
            UR = small_p.tile([128, D], BF16, tag="UR")
            nc.vector.tensor_copy(UR[:], UR8[:].bitcast(F8E4))
            nc.vector.tensor_copy(U_all[:, t * 64:(t + 1) * 64], UR[:])

            # --- gather x1 rows (fp8) from the table by item id ---------
            idx16 = small_p.tile([128, LI], U16, tag="idx16")
            nc.sync.dma_start(
                idx16[:],
                t_big[R_IHG + t * 128:R_IHG + (t + 1) * 128, :].bitcast(U16))
            idx32 = small_p.tile([128, LI], I32, tag="idx32")
            nc.vector.tensor_copy(idx32[:], idx16[:])
            x1f8 = big_p.tile([128, LI * D], U8, tag="x1f8")
            for i in range(LI):
                nc.gpsimd.indirect_dma_start(
                    out=x1f8[:, i * D:(i + 1) * D], out_offset=None,
                    in_=tab8,
                    in_offset=IndirectOffsetOnAxis(ap=idx32[:, i:i + 1],
                                                   axis=0))
            x1 = big_p.tile([128, LI * D], BF16, tag="x1")
            nc.vector.tensor_copy(x1[:], x1f8[:].bitcast(F8E4))

            # --- unpack aux nibbles: item 2j in low, 2j+1 in high -------
            aux4 = small_p.tile([128, LI // 2], U8, tag="aux4")
            nc.sync.dma_start(
                aux4[:],
                t_big[R_AUX + t * 32:R_AUX + (t + 1) * 32, :].bitcast(U8))
            aux8 = small_p.tile([128, LI], U8, tag="aux8")
            nc.vector.tensor_scalar(out=_stride2(aux8[:], 0, LI // 2),
                                    in0=aux4[:], scalar1=15, scalar2=None,
                                    op0=ALU.bitwise_and)
            nc.vector.tensor_scalar(out=_stride2(aux8[:], 1, LI // 2),
                                    in0=aux4[:], scalar1=4, scalar2=None,
                                    op0=ALU.logical_shift_right)
            r8 = small_p.tile([128, LI], U8, tag="r8")
            nc.vector.tensor_scalar(out=r8[:], in0=aux8[:], scalar1=1,
                                    scalar2=None,
                                    op0=ALU.logical_shift_right)
            rrf = small_p.tile([128, LI], BF16, tag="rrf")
            nc.vector.tensor_copy(rrf[:], r8[:])
            rtmp = big_p.tile([128, LI * D], BF16, tag="rtmp")
            for k in range(5):
                mk = small_p.tile([128, LI], BF16, tag="mk")
                nc.vector.tensor_scalar(out=mk[:], in0=rrf[:],
                                        scalar1=float(k), scalar2=None,
                                        op0=ALU.is_equal)
                nc.vector.tensor_tensor(
                    out=rtmp[:].rearrange("p (i f) -> p i f", f=64),
                    in0=_bcast_inner(mk[:], 64),
                    in1=_bcast_mid(rep5[:, k * D:(k + 1) * D], 64),
                    op=ALU.mult)
                nc.vector.tensor_tensor(out=x1[:], in0=x1[:], in1=rtmp[:],
                                        op=ALU.add)

            # --- mask bias rows: IMB = (mask - 1) * 1e9 -----------------
            mm8 = small_p.tile([128, LI], U8, tag="mm8")
            nc.vector.tensor_scalar(out=mm8[:], in0=aux8[:], scalar1=1,
                                    scalar2=None, op0=ALU.bitwise_and)
            mmf = small_p.tile([128, LI], F32, tag="mmf")
            nc.vector.tensor_copy(mmf[:], mm8[:])
            IMB = small_p.tile([128, LI], F32, tag="IMB")
            nc.vector.tensor_scalar(out=IMB[:], in0=mmf[:], scalar1=1.0,
                                    scalar2=1e9, op0=ALU.subtract,
                                    op1=ALU.mult)

            # x = relu(x1)   (row-major canonical x)
            xR = big_p.tile([128, LI * D], BF16, tag="xR")
            nc.scalar.activation(xR[:], x1[:], AF.Relu)

            # forward transposes -> feature-major packed pairs
            xT = big_p.tile([128, LI * D], BF16, tag="xT")
            for g in range(8):       # 8 groups of 4 item-pair blocks
                pst = ps_tr.tile([128, 512], BF16, tag="tr")
                for j in range(4):
                    blk = g * 4 + j
                    nc.tensor.transpose(
                        pst[:, j * 128:(j + 1) * 128],
                        xR[:, blk * 128:(blk + 1) * 128], idb)
                eng = nc.scalar if g % 2 == 0 else nc.vector
                if eng is nc.scalar:
                    nc.scalar.copy(xT[:, g * 512:(g + 1) * 512], pst[:])
                else:
                    nc.vector.tensor_copy(xT[:, g * 512:(g + 1) * 512],
                                          pst[:])

            # c_u = A1bot^T u + b1  (both halves)
            psU = ps_sm.tile([128, 128], BF16, tag="psU")
            nc.tensor.transpose(psU[0:64, :], UR[:, 0:64], idb)
            nc.tensor.transpose(psU[64:128, :], UR[:, 0:64], idb)
            UT2 = small_p.tile([128, 128], BF16, tag="UT2")
            nc.vector.tensor_copy(UT2[:], psU[:])
            psc = ps_sm.tile([128, 128], F32, tag="psc")
            nc.tensor.matmul(psc[0:64, :], lhsT=wsl("A1bot")[0:64, :],
                             rhs=UT2[0:64, :], start=True, stop=True)
            nc.tensor.matmul(psc[64:128, :], lhsT=wsl("A1bot")[64:128, :],
                             rhs=UT2[64:128, :], start=True, stop=True)
            cu = small_p.tile([128, 128], BF16, tag="cu")
            nc.vector.tensor_scalar(out=cu[:], in0=psc[:],
                                    scalar1=fbc("b1a2"), scalar2=None,
                                    op0=ALU.add)
            curep = small_p.tile([128, 512], BF16, tag="curep")
            nc.vector.tensor_copy(curep[:].rearrange("p (r q) -> p r q", r=4),
                                  _bcast_mid(cu[:], 4))

            AL = small_p.tile([128, LI], F32, tag="AL")
            h2a_full = big_p.tile([128, LI * D], BF16, tag="h2a")
            for c in range(8):
                sl = slice(c * 512, (c + 1) * 512)
                # --- att layer 1 (x part + u part via identity-matmul)
                ps1 = ps_mm.tile([128, 512], F32, tag="mm")
                nc.tensor.matmul(ps1[0:64, :], lhsT=wsl("A1top")[0:64, :],
                                 rhs=xT[0:64, sl], start=True, stop=False,
                                 skip_group_check=True)
                nc.tensor.matmul(ps1[64:128, :], lhsT=wsl("A1top")[64:128, :],
                                 rhs=xT[64:128, sl], start=True, stop=False,
                                 skip_group_check=True)
                nc.tensor.matmul(ps1[:], lhsT=idb, rhs=curep[:],
                                 start=False, stop=True,
                                 skip_group_check=True)
                h1 = chunk_p.tile([128, 512], BF16, tag="h1")
                nc.scalar.activation(h1[:], ps1[:], AF.Relu)
                # --- att layer 2 + bias row + (relu, * a3) on DVE
                ps2 = ps_mm.tile([128, 512], F32, tag="mm")
                nc.tensor.matmul(ps2[0:64, :], lhsT=wsl("A2")[0:64, :],
                                 rhs=h1[0:64, :], start=True, stop=False,
                                 skip_group_check=True)
                nc.tensor.matmul(ps2[64:128, :], lhsT=wsl("A2")[64:128, :],
                                 rhs=h1[64:128, :], start=True, stop=False,
                                 skip_group_check=True)
                nc.tensor.matmul(ps2[0:64, :], lhsT=rowv("b2a"),
                                 rhs=ones[:, 0:512], start=False, stop=False,
                                 skip_group_check=True)
                nc.tensor.matmul(ps2[64:128, :], lhsT=rowv("b2a"),
                                 rhs=ones[:, 0:512], start=False, stop=True,
                                 skip_group_check=True)
                h2a = h2a_full[:, sl]
                nc.vector.tensor_scalar(out=h2a, in0=ps2[:], scalar1=0.0,
                                        scalar2=fbc("a3pk"), op0=ALU.max,
                                        op1=ALU.mult)
                # --- att layer 3: back-transpose + grouped partition sum
                psb = ps_bk.tile([128, 512], BF16, tag="bk")
                for j in range(4):
                    nc.tensor.transpose(psb[:, j * 128:(j + 1) * 128],
                                        h2a[:, j * 128:(j + 1) * 128], idb)
                nc.vector.tensor_reduce(
                    out=AL[:, c * 8:(c + 1) * 8],
                    in_=psb[:].rearrange("p (i f) -> p i f", f=64),
                    op=ALU.add, axis=AXL.X)

            # --- masked softmax over items
            nc.vector.tensor_tensor(out=AL[:], in0=AL[:], in1=IMB[:],
                                    op=ALU.add)
            nmx = small_p.tile([128, 1], F32, tag="nmx")
            nc.vector.tensor_reduce(out=nmx[:], in_=AL[:], op=ALU.max,
                                    axis=AXL.X, negate=True)
            ex = small_p.tile([128, LI], BF16, tag="ex")
            nc.scalar.activation(ex[:], AL[:], AF.Exp, bias=nmx[:])
            sm = small_p.tile([128, 1], F32, tag="sm")
            nc.vector.tensor_reduce(out=sm[:], in_=ex[:], op=ALU.add,
                                    axis=AXL.X)
            rec = small_p.tile([128, 1], F32, tag="rec")
            nc.vector.reciprocal(rec[:], sm[:])

            # --- weighted sum over items (row-major)
            wtmp = big_p.tile([128, LI * D], BF16, tag="wtmp")
            nc.vector.tensor_tensor(
                out=wtmp[:].rearrange("p (i f) -> p i f", f=64),
                in0=xR[:].rearrange("p (i f) -> p i f", f=64),
                in1=_bcast_inner(ex[:], 64), op=ALU.mult)
            hIr = small_p.tile([128, D], F32, tag="hIr")
            nc.vector.tensor_reduce(
                out=hIr[:],
                in_=wtmp[:].rearrange("p (i f) -> p f i", f=64),
                op=ALU.add, axis=AXL.X)
            nc.vector.tensor_scalar(out=HI_all[:, t * 64:(t + 1) * 64],
                                    in0=hIr[:], scalar1=rec[:], scalar2=None,
                                    op0=ALU.mult)

        # ================= PHASE 2: i_ln2 / i_ln3 for all uids ==========
        hIT = pers_p.tile([128, NT2 * 64], BF16)
        uT = pers_p.tile([128, NT2 * 64], BF16)
        for b in range(NT2 // 2):
            pst = ps_tr.tile([128, 512], BF16, tag="tr")
            nc.tensor.transpose(pst[:, 0:128],
                                HI_all[:, b * 128:(b + 1) * 128], idb)
            nc.tensor.transpose(pst[:, 128:256],
                                U_all[:, b * 128:(b + 1) * 128], idb)
            nc.scalar.copy(hIT[:, b * 128:(b + 1) * 128], pst[:, 0:128])
            nc.scalar.copy(uT[:, b * 128:(b + 1) * 128], pst[:, 128:256])

        hI2T = pers_p.tile([128, NT2 * 64], BF16)
        HIFT = pers_p.tile([128, NT2 * 64], BF16)
        csizes = [512, 512, 128]
        off = 0
        for cs in csizes:
            sl = slice(off, off + cs)
            psl = ps_mm.tile([128, 512], F32, tag="mm")
            nc.tensor.matmul(psl[0:64, 0:cs], lhsT=wsl("Wl2")[0:64, :],
                             rhs=hIT[0:64, sl], start=True, stop=False,
                             skip_group_check=True)
            nc.tensor.matmul(psl[64:128, 0:cs], lhsT=wsl("Wl2")[64:128, :],
                             rhs=hIT[64:128, sl], start=True, stop=False,
                             skip_group_check=True)
            nc.tensor.matmul(psl[0:64, 0:cs], lhsT=rowv("b2l"),
                             rhs=ones[:, 0:cs], start=False, stop=False,
                             skip_group_check=True)
            nc.tensor.matmul(psl[64:128, 0:cs], lhsT=rowv("b2l"),
                             rhs=ones[:, 0:cs], start=False, stop=True,
                             skip_group_check=True)
            nc.scalar.activation(hI2T[:, sl], psl[:, 0:cs], AF.Relu)

            psf = ps_mm.tile([128, 512], F32, tag="mm")
            nc.tensor.matmul(psf[0:64, 0:cs], lhsT=wsl("Wl3bot")[0:64, :],
                             rhs=hI2T[0:64, sl], start=True, stop=False,
                             skip_group_check=True)
            nc.tensor.matmul(psf[64:128, 0:cs], lhsT=wsl("Wl3bot")[64:128, :],
                             rhs=hI2T[64:128, sl], start=True, stop=False,
                             skip_group_check=True)
            nc.tensor.matmul(psf[0:64, 0:cs], lhsT=wsl("Wl3top")[0:64, :],
                             rhs=uT[0:64, sl], start=False, stop=False,
                             skip_group_check=True)
            nc.tensor.matmul(psf[64:128, 0:cs], lhsT=wsl("Wl3top")[64:128, :],
                             rhs=uT[64:128, sl], start=False, stop=False,
                             skip_group_check=True)
            nc.tensor.matmul(psf[0:64, 0:cs], lhsT=rowv("b3l"),
                             rhs=ones[:, 0:cs], start=False, stop=False,
                             skip_group_check=True)
            nc.tensor.matmul(psf[64:128, 0:cs], lhsT=rowv("b3l"),
                             rhs=ones[:, 0:cs], start=False, stop=True,
                             skip_group_check=True)
            nc.scalar.activation(HIFT[:, sl], psf[:, 0:cs], AF.Relu)
            off += cs

        # write HIF rows to DRAM scratch (row-major)
        for b in range(NT2 // 2):
            psw = ps_tr.tile([128, 512], BF16, tag="tr")
            nc.tensor.transpose(psw[:, 0:128],
                                HIFT[:, b * 128:(b + 1) * 128], idb)
            hrow = small_p.tile([128, 128], BF16, tag="hrow")
            nc.vector.tensor_copy(hrow[:], psw[:, 0:128])
            nc.sync.dma_start(hif[(2 * b) * 128:(2 * b + 1) * 128, :],
                              hrow[:, 0:64])
            nc.sync.dma_start(hif[(2 * b + 1) * 128:(2 * b + 2) * 128, :],
                              hrow[:, 64:128])

        # ================= PHASE 3: social aggregation ==================
        Ys = pers_p.tile([BC, LS * D], BF16)
        nc.sync.dma_start(
            Ys[:],
            hif[:].rearrange("(b n) f -> b (n f)", n=LS)[2:2 + BC, :])

        YsT = pers_p.tile([128, 16 * 64], BF16)
        for g in range(4):
            pst = ps_tr.tile([128, 512], BF16, tag="tr")
            for j in range(4):
                blk = g * 4 + j
                nc.tensor.transpose(pst[0:128, j * 64:(j + 1) * 64],
                                    Ys[:, blk * 128:(blk + 1) * 128],
                                    idb[0:64, 0:64])
            nc.scalar.copy(YsT[:, g * 256:(g + 1) * 256], pst[:, 0:256])

        psU = ps_sm.tile([128, 128], BF16, tag="psU")
        nc.tensor.transpose(psU[0:64, 0:64], U_all[0:64, 0:64],
                            idb[0:64, 0:64])
        nc.tensor.transpose(psU[64:128, 0:64], U_all[0:64, 0:64],
                            idb[0:64, 0:64])
        UqT = small_p.tile([128, 64], BF16, tag="UqT")
        nc.vector.tensor_copy(UqT[:], psU[:, 0:64])
        pscq = ps_sm.tile([128, 128], F32, tag="psc")
        nc.tensor.matmul(pscq[0:64, 0:64], lhsT=wsl("sA1bot")[0:64, :],
                         rhs=UqT[0:64, :], start=True, stop=True)
        nc.tensor.matmul(pscq[64:128, 0:64], lhsT=wsl("sA1bot")[64:128, :],
                         rhs=UqT[64:128, :], start=True, stop=True)
        cq = small_p.tile([128, 64], BF16, tag="cq")
        nc.vector.tensor_scalar(out=cq[:], in0=pscq[:, 0:64],
                                scalar1=fbc("b1s2"), scalar2=None,
                                op0=ALU.add)
        cqrep = small_p.tile([128, 512], BF16, tag="curep")
        nc.vector.tensor_copy(cqrep[:].rearrange("p (r q) -> p r q", r=8),
                              _bcast_mid(cq[:], 8))

        ALs = small_p.tile([BC, LS], F32, tag="ALs")
        g2a_full = pers_p.tile([128, 16 * 64], BF16)
        for c in range(2):
            sl = slice(c * 512, (c + 1) * 512)
            ps1 = ps_mm.tile([128, 512], F32, tag="mm")
            nc.tensor.matmul(ps1[0:64, :], lhsT=wsl("sA1top")[0:64, :],
                             rhs=YsT[0:64, sl], start=True, stop=False,
                             skip_group_check=True)
            nc.tensor.matmul(ps1[64:128, :], lhsT=wsl("sA1top")[64:128, :],
                             rhs=YsT[64:128, sl], start=True, stop=False,
                             skip_group_check=True)
            nc.tensor.matmul(ps1[:], lhsT=idb, rhs=cqrep[:],
                             start=False, stop=True, skip_group_check=True)
            g1 = chunk_p.tile([128, 512], BF16, tag="h1")
            nc.scalar.activation(g1[:], ps1[:], AF.Relu)
            ps2 = ps_mm.tile([128, 512], F32, tag="mm")
            nc.tensor.matmul(ps2[0:64, :], lhsT=wsl("sA2")[0:64, :],
                             rhs=g1[0:64, :], start=True, stop=False,
                             skip_group_check=True)
            nc.tensor.matmul(ps2[64:128, :], lhsT=wsl("sA2")[64:128, :],
                             rhs=g1[64:128, :], start=True, stop=False,
                             skip_group_check=True)
            nc.tensor.matmul(ps2[0:64, :], lhsT=rowv("b2s"),
                             rhs=ones[:, 0:512], start=False, stop=False,
                             skip_group_check=True)
            nc.tensor.matmul(ps2[64:128, :], lhsT=rowv("b2s"),
                             rhs=ones[:, 0:512], start=False, stop=True,
                             skip_group_check=True)
            g2a = g2a_full[:, sl]
            nc.vector.tensor_scalar(out=g2a, in0=ps2[:], scalar1=0.0,
                                    scalar2=fbc("a3s2"), op0=ALU.max,
                                    op1=ALU.mult)
            for h in range(2):
                psb = ps_bk.tile([64, 512], BF16, tag="bk")
                for j in range(4):
                    blk = h * 4 + j
                    nc.tensor.transpose(psb[0:64, j * 128:(j + 1) * 128],
                                        g2a[:, blk * 64:(blk + 1) * 64], idb)
                nc.vector.tensor_reduce(
                    out=ALs[:, c * 16 + h * 8:c * 16 + (h + 1) * 8],
                    in_=psb[:].rearrange("p (i f) -> p i f", f=64),
                    op=ALU.add, axis=AXL.X)

        nc.vector.tensor_tensor(out=ALs[:], in0=ALs[:], in1=smb[:],
                                op=ALU.add)
        nmxs = small_p.tile([BC, 1], F32, tag="nmxs")
        nc.vector.tensor_reduce(out=nmxs[:], in_=ALs[:], op=ALU.max,
                                axis=AXL.X, negate=True)
        exs = small_p.tile([BC, LS], BF16, tag="exs")
        nc.scalar.activation(exs[:], ALs[:], AF.Exp, bias=nmxs[:])
        sms = small_p.tile([BC, 1], F32, tag="sms")
        nc.vector.tensor_reduce(out=sms[:], in_=exs[:], op=ALU.add,
                                axis=AXL.X)
        recs = small_p.tile([BC, 1], F32, tag="recs")
        nc.vector.reciprocal(recs[:], sms[:])

        wtmps = pers_p.tile([BC, LS * D], BF16)
        nc.vector.tensor_tensor(
            out=wtmps[:].rearrange("p (i f) -> p i f", f=64),
            in0=Ys[:].rearrange("p (i f) -> p i f", f=64),
            in1=_bcast_inner(exs[:], 64), op=ALU.mult)
        hSr = small_p.tile([BC, D], F32, tag="hSr")
        nc.vector.tensor_reduce(
            out=hSr[:], in_=wtmps[:].rearrange("p (i f) -> p f i", f=64),
            op=ALU.add, axis=AXL.X)
        hS = small_p.tile([BC, D], BF16, tag="hS")
        nc.vector.tensor_scalar(out=hS[:], in0=hSr[:], scalar1=recs[:],
                                scalar2=None, op0=ALU.mult)

        # s_ln1
        psh = ps_sm.tile([128, 128], BF16, tag="psU")
        nc.tensor.transpose(psh[0:64, 0:64], hS[:], idb[0:64, 0:64])
        hST = small_p.tile([64, 64], BF16, tag="hST")
        nc.vector.tensor_copy(hST[:], psh[0:64, 0:64])
        pl1 = ps_sm.tile([128, 128], F32, tag="psc")
        nc.tensor.matmul(pl1[0:64, 0:64], lhsT=wsl("sWl1")[0:64, :],
                         rhs=hST[:], start=True, stop=False,
                         skip_group_check=True)
        nc.tensor.matmul(pl1[0:64, 0:64], lhsT=rowv("b1ls"),
                         rhs=ones[:, 0:64], start=False, stop=True,
                         skip_group_check=True)
        hS2T = small_p.tile([64, 64], BF16, tag="hS2T")
        nc.scalar.activation(hS2T[:], pl1[0:64, 0:64], AF.Relu)

        # s_ln2 on concat(hI_self, hS2)
        pf = ps_sm.tile([128, 128], F32, tag="psc")
        nc.tensor.matmul(pf[0:64, 0:64], lhsT=wsl("sWl2top")[0:64, :],
                         rhs=HIFT[0:64, 0:64], start=True, stop=False,
                         skip_group_check=True)
        nc.tensor.matmul(pf[0:64, 0:64], lhsT=wsl("sWl2bot")[0:64, :],
                         rhs=hS2T[:], start=False, stop=False,
                         skip_group_check=True)
        nc.tensor.matmul(pf[0:64, 0:64], lhsT=rowv("b2ls"),
                         rhs=ones[:, 0:64], start=False, stop=True,
                         skip_group_check=True)
        fT = small_p.tile([64, 64], BF16, tag="fT")
        nc.scalar.activation(fT[:], pf[0:64, 0:64], AF.Relu)

        # s_ln3
        po = ps_sm.tile([128, 128], F32, tag="psc")
        nc.tensor.matmul(po[0:64, 0:64], lhsT=wsl("sWl3")[0:64, :],
                         rhs=fT[:], start=True, stop=False,
                         skip_group_check=True)
        nc.tensor.matmul(po[0:64, 0:64], lhsT=rowv("b3ls"),
                         rhs=ones[:, 0:64], start=False, stop=True,
                         skip_group_check=True)
        oT = small_p.tile([64, 64], F32, tag="oT")
        nc.scalar.activation(oT[:], po[0:64, 0:64], AF.Relu)

        # transpose to row-major, AllGather across cores, write out
        pout = ps_sm.tile([128, 128], F32, tag="psc")
        nc.tensor.transpose(pout[0:64, 0:64], oT[:], idf[0:64, 0:64])
        orow = small_p.tile([64, 64], F32, tag="orow")
        nc.vector.tensor_copy(orow[:], pout[0:64, 0:64])
        if full_tab:
            nc.sync.dma_start(t_out[0:BC, :], orow[:])
        else:
            ob = dram_p.tile([BC, D], F32, tag="ob")
            og = dram_p.tile([B, D], F32, tag="og", addr_space="Shared")
            nc.gpsimd.dma_start(ob[:], orow[:])
            nc.gpsimd.collective_compute(
                "AllGather", ALU.bypass,
                replica_groups=[list(range(NCORES))],
                ins=[ob.opt()], outs=[og.opt()])
            nc.gpsimd.dma_start(t_out[:], og[:])

    nc.compile()
    nc._replicated_out = not full_tab
    return nc


def _prep(inputs):
    """Host-side constant folding + dtype prep (shared across cores)."""
    f32 = np.float32
    item_w = np.asarray(inputs["item_w"], f32)
    rating_w = np.asarray(inputs["rating_w"], f32)
    user_w = np.asarray(inputs["user_w"], f32)
    W1 = np.asarray(inputs["i_ln1_w"], f32)
    b1 = np.asarray(inputs["i_ln1_b"], f32)

    f8 = ml_dtypes.float8_e4m3fn
    itw1 = (item_w @ W1[:64]).astype(f8)
    rtw1 = (rating_w @ W1[64:] + b1).astype(nbf)
    userb = user_w.astype(f8)
    ihist = np.asarray(inputs["item_hist"], np.int64)
    rhist = np.asarray(inputs["rating_hist"], np.int64)
    imask = np.asarray(inputs["item_mask"])

    def stack2(w):
        return np.vstack([w, w])

    wbcols = []
    i_att1 = np.asarray(inputs["i_att1_w"], f32)
    s_att1 = np.asarray(inputs["s_att1_w"], f32)
    s_ln2 = np.asarray(inputs["s_ln2_w"], f32)
    i_ln3 = np.asarray(inputs["i_ln3_w"], f32)
    for w in [i_att1[:64], i_att1[64:], np.asarray(inputs["i_att2_w"], f32),
              np.asarray(inputs["i_ln2_w"], f32), i_ln3[:64], i_ln3[64:],
              s_att1[:64], s_att1[64:], np.asarray(inputs["s_att2_w"], f32),
              np.asarray(inputs["s_ln1_w"], f32), s_ln2[:64], s_ln2[64:],
              np.asarray(inputs["s_ln3_w"], f32)]:
        wbcols.append(stack2(w))
    wbcols.append(np.eye(128, dtype=f32))
    wbmat = np.concatenate(wbcols, axis=1).astype(nbf)

    rowsv = np.concatenate(
        [np.asarray(inputs[k], f32) for k in
         ["i_att2_b", "i_ln2_b", "i_ln3_b", "s_att2_b", "s_ln1_b",
          "s_ln2_b", "s_ln3_b"]])[None, :].astype(nbf)

    fbm = np.zeros((128, 4), f32)
    fbm[:, 0] = np.tile(np.asarray(inputs["i_att1_b"], f32), 2)
    fbm[:, 1] = np.tile(np.asarray(inputs["i_att3_w"], f32)[:, 0], 2)
    fbm[:, 2] = np.tile(np.asarray(inputs["s_att1_b"], f32), 2)
    fbm[:, 3] = np.tile(np.asarray(inputs["s_att3_w"], f32)[:, 0], 2)

    # wb blob as rows-of-64 (p-major), appended per-core to the AG shard
    wbrows = wbmat.reshape(128, 15, 64).reshape(1920, 64)

    # item id -> fp8-row slot in the AllGather'd table (skip wb appendix)
    def agmap(ids):
        return (ids + (FP8SH - SHROWS) * (ids // SHROWS)).astype(np.uint16)

    def b128(a):
        v = np.ascontiguousarray(a).view(np.uint8).reshape(-1)
        assert v.size % 128 == 0, v.size
        return v.reshape(-1, 128)

    nodes = np.asarray(inputs["nodes"], np.int64)
    shist = np.asarray(inputs["social_hist"], np.int64)
    smask = np.asarray(inputs["social_mask"])
    in_maps = []
    for c in range(NCORES):
        nd = nodes[c * BC:(c + 1) * BC]
        nbrs = shist[nd]                           # [BC, LS]
        uids = np.zeros(NUID, np.int64)
        uids[:BC] = nd
        uids[BC:BC + BC * LS] = nbrs.reshape(-1)
        smb = np.where(smask[nd], 0.0, -1e9).astype(f32)
        aux = (2 * rhist[uids] + imask[uids]).astype(np.uint8)
        aux4 = (aux[:, 0::2] | (aux[:, 1::2] << 4)).astype(np.uint8)
        blob = np.concatenate([
            b128(itw1[c * SHROWS:(c + 1) * SHROWS]),
            b128(wbrows[c * WBROWS:(c + 1) * WBROWS]),
            b128(userb[uids]),
            b128(agmap(ihist[uids])),
            b128(aux4),
            b128(rowsv),
            b128(rtw1.reshape(1, 5 * D)),
            b128(smb),
            b128(fbm),
        ], axis=0)
        assert blob.shape[0] == RTOT, blob.shape
        in_maps.append({"big": blob.view(nbf).reshape(RTOT, D)})
    return in_maps


def _prep_sim0(inputs):
    """Single-core (core 0) input map for CoreSim with full_tab=True."""
    maps = _prep(inputs)
    m = dict(maps[0])
    m["tabf"] = np.concatenate(
        [np.asarray(mp["big"])[:AGROWS] for mp in maps], axis=0)
    return m


def kernel(**inputs):
    if "nc" not in _CACHE:
        _CACHE["nc"] = build_program()
    nc = _CACHE["nc"]
    in_maps = _prep(inputs)
    res = run_bass_kernel_spmd(nc, in_maps, core_ids=list(range(NCORES)))
    return np.asarray(res.results[0]["out"]).astype(np.float32)



# revision 5
# speedup vs baseline: 91.6877x; 91.6877x over previous
"""Trainium2 Bass kernel for nn_Aggre_social (GNN message passing, social rec).

Strategy: data-parallel over the B=512 query users across 8 NeuronCores
(64 queries/core).  Per core we compute item_final() for 64 self uids +
64*32 = 2048 neighbor uids = 2112 uids (padded to 17 tiles of 128), each
with a 64-item history.

v3 layout (device-time optimized):
  - the folded item table itw1 = item_w @ i_ln1_w[:64] is staged fp8
    REPLICATED to every core, so there is no on-device AllGather;
  - item embedding rows are gathered on device with per-column indirect
    DMA ([128,1] indices -- the only indirect form this HW supports;
    batched multi-index offsets scramble and the dma_gather extended
    instruction is absent from the bedrock ucode image);
  - the 5-row rating table rtw1 = rating_w @ i_ln1_w[64:] + b1 is added
    via 5 select-multiply-add vector ops per tile (hides under the
    Pool-engine gather bottleneck);
  - per-(uid,item) item ids ship as uint16 rows, ratings and masks as
    packed 4-bit aux rows; user rows are host-gathered fp8;
  - each core writes only its own [64,64] output shard; the host
    concatenates (no output AllGather).
Compute is done mostly in bf16 with a feature-major ("transposed")
layout for the attention MLPs and a row-major layout for softmax /
weighted sums.
"""

import sys

sys.path.insert(0, "/opt/trn_rl_repo")

import numpy as np
import ml_dtypes
from contextlib import ExitStack

import concourse.bass as bass
import concourse.bacc as bacc
import concourse.tile as tile
from concourse import mybir
from concourse.bass import IndirectOffsetOnAxis, AP
from concourse.bass_utils import run_bass_kernel_spmd

BF16 = mybir.dt.bfloat16
F32 = mybir.dt.float32
I32 = mybir.dt.int32
U16 = mybir.dt.uint16
U8 = mybir.dt.uint8
F8E4 = mybir.dt.float8e4
AF = mybir.ActivationFunctionType
ALU = mybir.AluOpType
AXL = mybir.AxisListType

NCORES = 8
B = 512
BC = B // NCORES          # 64 queries per core
LS = 32
LI = 64
D = 64
NU = 50000
NTAB8 = 50048             # fp8 itw1 rows (64B each), padded
NT = 17                   # uid tiles per core (128 uids each)
NUID = NT * 128           # 2176 (2112 real + 64 pad)
NT2 = 18                  # padded to even for pair-blocking in phase 2
HIFROWS = NT2 * 128       # 2304 rows in HIF scratch

# blob region offsets, in 64-elem bf16 rows (128B each)
R_WB = 0                  # wb blob: 1920 rows
R_USER = 1920             # fp8 user rows: 2176 x 64B = 1088 rows
R_IHG = R_USER + 1088     # u16 item ids: 2176 rows
R_AUX = R_IHG + 2176      # 4-bit packed 2*rating+mask: 2176x32B = 544 rows
R_ROWS = R_AUX + 544      # bias row-vectors: 7 rows
R_RT = R_ROWS + 7         # folded rating rows: 5 rows
R_SMB = R_RT + 5          # social mask bias f32 [64,32]: 64 rows
R_FB = R_SMB + 64         # per-partition f32 bias/scale cols [128,4]: 16
RTOT = R_FB + 16          # 5820 rows = 745KB per core

nbf = ml_dtypes.bfloat16

_CACHE = {}


# ---------------------------------------------------------------------------
# run_bass_via_pjrt rebuilds its jit closure (and so retraces + relowers the
# whole program) on every call, which costs ~0.5s/dispatch of pure client-side
# work.  Memoize the jitted sharded callable per Bass program: semantics are
# identical (inputs are still concatenated, staged to the devices, executed
# and fetched on every call) -- only the jax trace/lower step is reused.
# run_bass_kernel_spmd remains the dispatch entry point.
# ---------------------------------------------------------------------------
_PJRT_JIT_CACHE = {}


def _install_pjrt_cache():
    from concourse import bass2jax as b2j
    import jax
    from jax.sharding import Mesh, PartitionSpec
    from jax.experimental.shard_map import shard_map

    orig = b2j.run_bass_via_pjrt
    if getattr(orig, "_aggre_cached", False):
        return

    def cached_run(nc, in_maps, n_cores):
        key = (id(nc), n_cores)
        if key not in _PJRT_JIT_CACHE:
            if nc.dbg_addr is not None or n_cores == 1:
                return orig(nc, in_maps, n_cores)  # uncommon paths: passthru
            b2j.install_neuronx_cc_hook()
            partition_name = (nc.partition_id_tensor.name
                              if nc.partition_id_tensor else None)
            in_names, out_names, out_avals, zero_shapes = [], [], [], []
            for alloc in nc.m.functions[0].allocations:
                if not isinstance(alloc, mybir.MemoryLocationSet):
                    continue
                name = alloc.memorylocations[0].name
                if alloc.kind == "ExternalInput":
                    if name != partition_name:
                        in_names.append(name)
                elif alloc.kind == "ExternalOutput":
                    out_names.append(name)
                    shape = tuple(alloc.tensor_shape)
                    dtype = mybir.dt.np(alloc.dtype)
                    out_avals.append(jax.core.ShapedArray(shape, dtype))
                    zero_shapes.append((shape, dtype))
            n_params = len(in_names)
            all_names = list(in_names) + out_names
            if partition_name is not None:
                all_names.append(partition_name)
            donate = tuple(range(n_params, n_params + len(out_names)))

            def _body(*args):
                operands = list(args)
                if partition_name is not None:
                    operands.append(b2j.partition_id_tensor())
                outs = b2j._bass_exec_p.bind(
                    *operands, out_avals=tuple(out_avals),
                    in_names=tuple(all_names), out_names=tuple(out_names),
                    lowering_input_output_aliases=(),
                    sim_require_finite=True, sim_require_nnan=True, nc=nc)
                return tuple(outs)

            mesh = Mesh(np.asarray(jax.devices()[:n_cores]), ("core",))
            sharded = jax.jit(
                shard_map(_body, mesh=mesh,
                          in_specs=(PartitionSpec("core"),)
                          * (n_params + len(out_names)),
                          out_specs=(PartitionSpec("core"),)
                          * len(out_names), check_rep=False),
                donate_argnums=donate, keep_unused=True)
            _PJRT_JIT_CACHE[key] = (nc, sharded, in_names, out_names,
                                    out_avals, zero_shapes)
        (_, sharded, in_names, out_names, out_avals,
         zero_shapes) = _PJRT_JIT_CACHE[key]
        n_params = len(in_names)
        concat_in = [
            np.concatenate([np.asarray(in_maps[c][name])
                            for c in range(n_cores)], axis=0)
            for name in in_names]
        # donate the previous call's device-resident output buffers (the
        # kernel overwrites every output element); zeros on the first call
        prev = _PJRT_JIT_CACHE.get(("prev_out", key))
        if prev is None:
            prev = [np.zeros((n_cores * s[0], *s[1:]), dt)
                    for s, dt in zero_shapes]
        out_arrs = sharded(*concat_in, *prev)
        _PJRT_JIT_CACHE[("prev_out", key)] = list(out_arrs)
        return [
            {name: np.asarray(out_arrs[i]).reshape(
                n_cores, *out_avals[i].shape)[c]
             for i, name in enumerate(out_names)}
            for c in range(n_cores)]

    cached_run._aggre_cached = True
    b2j.run_bass_via_pjrt = cached_run


_install_pjrt_cache()


def _bcast_mid(ap, rep):
    """[P, n] AP -> [P, rep, n] AP with stride-0 middle dim."""
    assert len(ap.ap) == 2
    return AP(ap.tensor, ap.offset, [ap.ap[0], [0, rep], ap.ap[1]])


def _bcast_inner(ap, rep):
    """[P, n] AP -> [P, n, rep] AP with stride-0 inner dim."""
    assert len(ap.ap) == 2
    return AP(ap.tensor, ap.offset, [ap.ap[0], ap.ap[1], [0, rep]])


def _stride2(ap, phase, n):
    """[P, 2n] AP -> [P, n] AP over elements phase, phase+2, ..."""
    assert len(ap.ap) == 2
    return AP(ap.tensor, ap.offset + phase, [ap.ap[0], [2, n]])


def build_program():
    nc = bacc.Bacc("TRN2", target_bir_lowering=False, debug=False,
                   num_devices=NCORES)

    # ---- DRAM I/O ------------------------------------------------------
    def din(name, shape, dt):
        return nc.dram_tensor(name, shape, dt, kind="ExternalInput").ap()

    t_tab = din("tab", [NTAB8, D], U8)       # fp8 itw1 rows (64B each)
    t_big = din("big", [RTOT, D], BF16)      # everything else (see R_*)
    t_out = nc.dram_tensor("out", [BC, D], F32, kind="ExternalOutput").ap()

    W = {}  # weight blob column slots
    for i, name in enumerate(["A1top", "A1bot", "A2", "Wl2", "Wl3top",
                              "Wl3bot", "sA1top", "sA1bot", "sA2", "sWl1",
                              "sWl2top", "sWl2bot", "sWl3"]):
        W[name] = i * 64
    IDB_OFF = 13 * 64
    RW = {n: i * 64 for i, n in enumerate(
        ["b2a", "b2l", "b3l", "b2s", "b1ls", "b2ls", "b3ls"])}
    FB = {"b1a2": 0, "a3pk": 1, "b1s2": 2, "a3s2": 3}

    with tile.TileContext(nc) as tc, ExitStack() as ctx:
        const_p = ctx.enter_context(tc.tile_pool(name="const", bufs=1))
        small_p = ctx.enter_context(tc.tile_pool(name="small", bufs=3))
        big_p = ctx.enter_context(tc.tile_pool(name="big", bufs=2))
        chunk_p = ctx.enter_context(tc.tile_pool(name="chunk", bufs=4))
        pers_p = ctx.enter_context(tc.tile_pool(name="pers", bufs=1))
        dram_p = ctx.enter_context(tc.tile_pool(name="dram", bufs=1,
                                                space="DRAM"))
        ps_tr = ctx.enter_context(tc.tile_pool(name="ps_tr", bufs=2,
                                               space="PSUM"))
        ps_mm = ctx.enter_context(tc.tile_pool(name="ps_mm", bufs=2,
                                               space="PSUM"))
        ps_bk = ctx.enter_context(tc.tile_pool(name="ps_bk", bufs=2,
                                               space="PSUM"))
        ps_sm = ctx.enter_context(tc.tile_pool(name="ps_sm", bufs=1,
                                               space="PSUM"))

        # ---- constants into SBUF --------------------------------------
        # wb rows are p-major: partition p holds rows [15p, 15p+15)
        wb = const_p.tile([128, 13 * 64 + 128], BF16)
        nc.sync.dma_start(
            wb[:],
            t_big[R_WB:R_WB + 1920, :].rearrange("(p j) f -> p (j f)", p=128))
        rows = const_p.tile([1, 7 * 64], BF16)
        nc.sync.dma_start(rows[:], t_big[R_ROWS:R_ROWS + 7, :])
        fb = const_p.tile([128, 4], F32)
        nc.sync.dma_start(fb[:], t_big[R_FB:R_FB + 16, :].bitcast(F32))
        smb = const_p.tile([BC, LS], F32)
        nc.sync.dma_start(smb[:], t_big[R_SMB:R_SMB + 64, :].bitcast(F32))
        ones = const_p.tile([1, 512], BF16)
        nc.vector.memset(ones[:], 1.0)
        idb = wb[:, IDB_OFF:IDB_OFF + 128]      # bf16 identity
        idf = const_p.tile([128, 128], F32)     # f32 identity (0/1 exact)
        nc.vector.tensor_copy(idf[:], idb)

        # rating rows -> broadcast to all 128 partitions: rep5[p, k*64+f]
        rtrow = const_p.tile([1, 5 * D], BF16)
        nc.sync.dma_start(rtrow[:], t_big[R_RT:R_RT + 5, :])
        ps_r5 = ps_mm.tile([128, 5 * D], F32, tag="mm")
        nc.tensor.matmul(ps_r5[:], lhsT=ones[:, 0:128], rhs=rtrow[:],
                         start=True, stop=True)
        rep5 = const_p.tile([128, 5 * D], BF16)
        nc.vector.tensor_copy(rep5[:], ps_r5[:])

        def wsl(name):
            return wb[:, W[name]:W[name] + 64]

        def rowv(name):
            return rows[:, RW[name]:RW[name] + 64]

        def fbc(name):
            return fb[:, FB[name]:FB[name] + 1]

        HI_all = pers_p.tile([128, NT2 * 64], BF16)
        U_all = pers_p.tile([128, NT2 * 64], BF16)
        nc.vector.memset(HI_all[:, NT * 64:], 0.0)
        nc.vector.memset(U_all[:, NT * 64:], 0.0)
        hif = dram_p.tile([HIFROWS, D], BF16, tag="hif")
        assert hif[:].offset == 0

        # ================= PHASE 1: item_final per uid-tile =============
        for t in range(NT):
            # user rows arrive fp8; widen to bf16 once
            UR8 = small_p.tile([128, D], U8, tag="UR8")
            nc.sync.dma_start(
                UR8[:],
                t_big[R_USER + t * 64:R_USER + (t + 1) * 64, :].bitcast(U8))
            UR = small_p.tile([128, D], BF16, tag="UR")
            nc.vector.tensor_copy(UR[:], UR8[:].bitcast(F8E4))
            nc.vector.tensor_copy(U_all[:, t * 64:(t + 1) * 64], UR[:])

            # --- gather x1 rows (fp8) from the table by item id ---------
            idx16 = small_p.tile([128, LI], U16, tag="idx16")
            nc.sync.dma_start(
                idx16[:],
                t_big[R_IHG + t * 128:R_IHG + (t + 1) * 128, :].bitcast(U16))
            idx32 = small_p.tile([128, LI], I32, tag="idx32")
            nc.vector.tensor_copy(idx32[:], idx16[:])
            x1f8 = big_p.tile([128, LI * D], U8, tag="x1f8")
            for i in range(LI):
                nc.gpsimd.indirect_dma_start(
                    out=x1f8[:, i * D:(i + 1) * D], out_offset=None,
                    in_=t_tab,
                    in_offset=IndirectOffsetOnAxis(ap=idx32[:, i:i + 1],
                                                   axis=0))
            x1 = big_p.tile([128, LI * D], BF16, tag="x1")
            nc.vector.tensor_copy(x1[:], x1f8[:].bitcast(F8E4))

            # --- unpack aux nibbles: item 2j in low, 2j+1 in high -------
            aux4 = small_p.tile([128, LI // 2], U8, tag="aux4")
            nc.sync.dma_start(
                aux4[:],
                t_big[R_AUX + t * 32:R_AUX + (t + 1) * 32, :].bitcast(U8))
            aux8 = small_p.tile([128, LI], U8, tag="aux8")
            nc.vector.tensor_scalar(out=_stride2(aux8[:], 0, LI // 2),
                                    in0=aux4[:], scalar1=15, scalar2=None,
                                    op0=ALU.bitwise_and)
            nc.vector.tensor_scalar(out=_stride2(aux8[:], 1, LI // 2),
                                    in0=aux4[:], scalar1=4, scalar2=None,
                                    op0=ALU.logical_shift_right)
            r8 = small_p.tile([128, LI], U8, tag="r8")
            nc.vector.tensor_scalar(out=r8[:], in0=aux8[:], scalar1=1,
                                    scalar2=None,
                                    op0=ALU.logical_shift_right)
            rrf = small_p.tile([128, LI], BF16, tag="rrf")
            nc.vector.tensor_copy(rrf[:], r8[:])
            rtmp = big_p.tile([128, LI * D], BF16, tag="rtmp")
            for k in range(5):
                mk = small_p.tile([128, LI], BF16, tag="mk")
                nc.vector.tensor_scalar(out=mk[:], in0=rrf[:],
                                        scalar1=float(k), scalar2=None,
                                        op0=ALU.is_equal)
                nc.vector.tensor_tensor(
                    out=rtmp[:].rearrange("p (i f) -> p i f", f=64),
                    in0=_bcast_inner(mk[:], 64),
                    in1=_bcast_mid(rep5[:, k * D:(k + 1) * D], 64),
                    op=ALU.mult)
                nc.vector.tensor_tensor(out=x1[:], in0=x1[:], in1=rtmp[:],
                                        op=ALU.add)

            # --- mask bias rows: IMB = (mask - 1) * 1e9 -----------------
            mm8 = small_p.tile([128, LI], U8, tag="mm8")
            nc.vector.tensor_scalar(out=mm8[:], in0=aux8[:], scalar1=1,
                                    scalar2=None, op0=ALU.bitwise_and)
            mmf = small_p.tile([128, LI], F32, tag="mmf")
            nc.vector.tensor_copy(mmf[:], mm8[:])
            IMB = small_p.tile([128, LI], F32, tag="IMB")
            nc.vector.tensor_scalar(out=IMB[:], in0=mmf[:], scalar1=1.0,
                                    scalar2=1e9, op0=ALU.subtract,
                                    op1=ALU.mult)

            # x = relu(x1)   (row-major canonical x)
            xR = big_p.tile([128, LI * D], BF16, tag="xR")
            nc.scalar.activation(xR[:], x1[:], AF.Relu)

            # forward transposes -> feature-major packed pairs
            xT = big_p.tile([128, LI * D], BF16, tag="xT")
            for g in range(8):       # 8 groups of 4 item-pair blocks
                pst = ps_tr.tile([128, 512], BF16, tag="tr")
                for j in range(4):
                    blk = g * 4 + j
                    nc.tensor.transpose(
                        pst[:, j * 128:(j + 1) * 128],
                        xR[:, blk * 128:(blk + 1) * 128], idb)
                eng = nc.scalar if g % 2 == 0 else nc.vector
                if eng is nc.scalar:
                    nc.scalar.copy(xT[:, g * 512:(g + 1) * 512], pst[:])
                else:
                    nc.vector.tensor_copy(xT[:, g * 512:(g + 1) * 512],
                                          pst[:])

            # c_u = A1bot^T u + b1  (both halves)
            psU = ps_sm.tile([128, 128], BF16, tag="psU")
            nc.tensor.transpose(psU[0:64, :], UR[:, 0:64], idb)
            nc.tensor.transpose(psU[64:128, :], UR[:, 0:64], idb)
            UT2 = small_p.tile([128, 128], BF16, tag="UT2")
            nc.vector.tensor_copy(UT2[:], psU[:])
            psc = ps_sm.tile([128, 128], F32, tag="psc")
            nc.tensor.matmul(psc[0:64, :], lhsT=wsl("A1bot")[0:64, :],
                             rhs=UT2[0:64, :], start=True, stop=True)
            nc.tensor.matmul(psc[64:128, :], lhsT=wsl("A1bot")[64:128, :],
                             rhs=UT2[64:128, :], start=True, stop=True)
            cu = small_p.tile([128, 128], BF16, tag="cu")
            nc.vector.tensor_scalar(out=cu[:], in0=psc[:],
                                    scalar1=fbc("b1a2"), scalar2=None,
                                    op0=ALU.add)
            curep = small_p.tile([128, 512], BF16, tag="curep")
            nc.vector.tensor_copy(curep[:].rearrange("p (r q) -> p r q", r=4),
                                  _bcast_mid(cu[:], 4))

            AL = small_p.tile([128, LI], F32, tag="AL")
            h2a_full = big_p.tile([128, LI * D], BF16, tag="h2a")
            for c in range(8):
                sl = slice(c * 512, (c + 1) * 512)
                # --- att layer 1 (x part + u part via identity-matmul)
                ps1 = ps_mm.tile([128, 512], F32, tag="mm")
                nc.tensor.matmul(ps1[0:64, :], lhsT=wsl("A1top")[0:64, :],
                                 rhs=xT[0:64, sl], start=True, stop=False,
                                 skip_group_check=True)
                nc.tensor.matmul(ps1[64:128, :], lhsT=wsl("A1top")[64:128, :],
                                 rhs=xT[64:128, sl], start=True, stop=False,
                                 skip_group_check=True)
                nc.tensor.matmul(ps1[:], lhsT=idb, rhs=curep[:],
                                 start=False, stop=True,
                                 skip_group_check=True)
                h1 = chunk_p.tile([128, 512], BF16, tag="h1")
                nc.scalar.activation(h1[:], ps1[:], AF.Relu)
                # --- att layer 2 + bias row + (relu, * a3) on DVE
                ps2 = ps_mm.tile([128, 512], F32, tag="mm")
                nc.tensor.matmul(ps2[0:64, :], lhsT=wsl("A2")[0:64, :],
                                 rhs=h1[0:64, :], start=True, stop=False,
                                 skip_group_check=True)
                nc.tensor.matmul(ps2[64:128, :], lhsT=wsl("A2")[64:128, :],
                                 rhs=h1[64:128, :], start=True, stop=False,
                                 skip_group_check=True)
                nc.tensor.matmul(ps2[0:64, :], lhsT=rowv("b2a"),
                                 rhs=ones[:, 0:512], start=False, stop=False,
                                 skip_group_check=True)
                nc.tensor.matmul(ps2[64:128, :], lhsT=rowv("b2a"),
                                 rhs=ones[:, 0:512], start=False, stop=True,
                                 skip_group_check=True)
                h2a = h2a_full[:, sl]
                nc.vector.tensor_scalar(out=h2a, in0=ps2[:], scalar1=0.0,
                                        scalar2=fbc("a3pk"), op0=ALU.max,
                                        op1=ALU.mult)
                # --- att layer 3: back-transpose + grouped partition sum
                psb = ps_bk.tile([128, 512], BF16, tag="bk")
                for j in range(4):
                    nc.tensor.transpose(psb[:, j * 128:(j + 1) * 128],
                                        h2a[:, j * 128:(j + 1) * 128], idb)
                nc.vector.tensor_reduce(
                    out=AL[:, c * 8:(c + 1) * 8],
                    in_=psb[:].rearrange("p (i f) -> p i f", f=64),
                    op=ALU.add, axis=AXL.X)

            # --- masked softmax over items
            nc.vector.tensor_tensor(out=AL[:], in0=AL[:], in1=IMB[:],
                                    op=ALU.add)
            nmx = small_p.tile([128, 1], F32, tag="nmx")
            nc.vector.tensor_reduce(out=nmx[:], in_=AL[:], op=ALU.max,
                                    axis=AXL.X, negate=True)
            ex = small_p.tile([128, LI], BF16, tag="ex")
            nc.scalar.activation(ex[:], AL[:], AF.Exp, bias=nmx[:])
            sm = small_p.tile([128, 1], F32, tag="sm")
            nc.vector.tensor_reduce(out=sm[:], in_=ex[:], op=ALU.add,
                                    axis=AXL.X)
            rec = small_p.tile([128, 1], F32, tag="rec")
            nc.vector.reciprocal(rec[:], sm[:])

            # --- weighted sum over items (row-major)
            wtmp = big_p.tile([128, LI * D], BF16, tag="wtmp")
            nc.vector.tensor_tensor(
                out=wtmp[:].rearrange("p (i f) -> p i f", f=64),
                in0=xR[:].rearrange("p (i f) -> p i f", f=64),
                in1=_bcast_inner(ex[:], 64), op=ALU.mult)
            hIr = small_p.tile([128, D], F32, tag="hIr")
            nc.vector.tensor_reduce(
                out=hIr[:],
                in_=wtmp[:].rearrange("p (i f) -> p f i", f=64),
                op=ALU.add, axis=AXL.X)
            nc.vector.tensor_scalar(out=HI_all[:, t * 64:(t + 1) * 64],
                                    in0=hIr[:], scalar1=rec[:], scalar2=None,
                                    op0=ALU.mult)

        # ================= PHASE 2: i_ln2 / i_ln3 for all uids ==========
        hIT = pers_p.tile([128, NT2 * 64], BF16)
        uT = pers_p.tile([128, NT2 * 64], BF16)
        for b in range(NT2 // 2):
            pst = ps_tr.tile([128, 512], BF16, tag="tr")
            nc.tensor.transpose(pst[:, 0:128],
                                HI_all[:, b * 128:(b + 1) * 128], idb)
            nc.tensor.transpose(pst[:, 128:256],
                                U_all[:, b * 128:(b + 1) * 128], idb)
            nc.scalar.copy(hIT[:, b * 128:(b + 1) * 128], pst[:, 0:128])
            nc.scalar.copy(uT[:, b * 128:(b + 1) * 128], pst[:, 128:256])

        hI2T = pers_p.tile([128, NT2 * 64], BF16)
        HIFT = pers_p.tile([128, NT2 * 64], BF16)
        csizes = [512, 512, 128]
        off = 0
        for cs in csizes:
            sl = slice(off, off + cs)
            psl = ps_mm.tile([128, 512], F32, tag="mm")
            nc.tensor.matmul(psl[0:64, 0:cs], lhsT=wsl("Wl2")[0:64, :],
                             rhs=hIT[0:64, sl], start=True, stop=False,
                             skip_group_check=True)
            nc.tensor.matmul(psl[64:128, 0:cs], lhsT=wsl("Wl2")[64:128, :],
                             rhs=hIT[64:128, sl], start=True, stop=False,
                             skip_group_check=True)
            nc.tensor.matmul(psl[0:64, 0:cs], lhsT=rowv("b2l"),
                             rhs=ones[:, 0:cs], start=False, stop=False,
                             skip_group_check=True)
            nc.tensor.matmul(psl[64:128, 0:cs], lhsT=rowv("b2l"),
                             rhs=ones[:, 0:cs], start=False, stop=True,
                             skip_group_check=True)
            nc.scalar.activation(hI2T[:, sl], psl[:, 0:cs], AF.Relu)

            psf = ps_mm.tile([128, 512], F32, tag="mm")
            nc.tensor.matmul(psf[0:64, 0:cs], lhsT=wsl("Wl3bot")[0:64, :],
                             rhs=hI2T[0:64, sl], start=True, stop=False,
                             skip_group_check=True)
            nc.tensor.matmul(psf[64:128, 0:cs], lhsT=wsl("Wl3bot")[64:128, :],
                             rhs=hI2T[64:128, sl], start=True, stop=False,
                             skip_group_check=True)
            nc.tensor.matmul(psf[0:64, 0:cs], lhsT=wsl("Wl3top")[0:64, :],
                             rhs=uT[0:64, sl], start=False, stop=False,
                             skip_group_check=True)
            nc.tensor.matmul(psf[64:128, 0:cs], lhsT=wsl("Wl3top")[64:128, :],
                             rhs=uT[64:128, sl], start=False, stop=False,
                             skip_group_check=True)
            nc.tensor.matmul(psf[0:64, 0:cs], lhsT=rowv("b3l"),
                             rhs=ones[:, 0:cs], start=False, stop=False,
                             skip_group_check=True)
            nc.tensor.matmul(psf[64:128, 0:cs], lhsT=rowv("b3l"),
                             rhs=ones[:, 0:cs], start=False, stop=True,
                             skip_group_check=True)
            nc.scalar.activation(HIFT[:, sl], psf[:, 0:cs], AF.Relu)
            off += cs

        # write HIF rows to DRAM scratch (row-major)
        for b in range(NT2 // 2):
            psw = ps_tr.tile([128, 512], BF16, tag="tr")
            nc.tensor.transpose(psw[:, 0:128],
                                HIFT[:, b * 128:(b + 1) * 128], idb)
            hrow = small_p.tile([128, 128], BF16, tag="hrow")
            nc.vector.tensor_copy(hrow[:], psw[:, 0:128])
            nc.sync.dma_start(hif[(2 * b) * 128:(2 * b + 1) * 128, :],
                              hrow[:, 0:64])
            nc.sync.dma_start(hif[(2 * b + 1) * 128:(2 * b + 2) * 128, :],
                              hrow[:, 64:128])

        # ================= PHASE 3: social aggregation ==================
        Ys = pers_p.tile([BC, LS * D], BF16)
        nc.sync.dma_start(
            Ys[:],
            hif[:].rearrange("(b n) f -> b (n f)", n=LS)[2:2 + BC, :])

        YsT = pers_p.tile([128, 16 * 64], BF16)
        for g in range(4):
            pst = ps_tr.tile([128, 512], BF16, tag="tr")
            for j in range(4):
                blk = g * 4 + j
                nc.tensor.transpose(pst[0:128, j * 64:(j + 1) * 64],
                                    Ys[:, blk * 128:(blk + 1) * 128],
                                    idb[0:64, 0:64])
            nc.scalar.copy(YsT[:, g * 256:(g + 1) * 256], pst[:, 0:256])

        psU = ps_sm.tile([128, 128], BF16, tag="psU")
        nc.tensor.transpose(psU[0:64, 0:64], U_all[0:64, 0:64],
                            idb[0:64, 0:64])
        nc.tensor.transpose(psU[64:128, 0:64], U_all[0:64, 0:64],
                            idb[0:64, 0:64])
        UqT = small_p.tile([128, 64], BF16, tag="UqT")
        nc.vector.tensor_copy(UqT[:], psU[:, 0:64])
        pscq = ps_sm.tile([128, 128], F32, tag="psc")
        nc.tensor.matmul(pscq[0:64, 0:64], lhsT=wsl("sA1bot")[0:64, :],
                         rhs=UqT[0:64, :], start=True, stop=True)
        nc.tensor.matmul(pscq[64:128, 0:64], lhsT=wsl("sA1bot")[64:128, :],
                         rhs=UqT[64:128, :], start=True, stop=True)
        cq = small_p.tile([128, 64], BF16, tag="cq")
        nc.vector.tensor_scalar(out=cq[:], in0=pscq[:, 0:64],
                                scalar1=fbc("b1s2"), scalar2=None,
                                op0=ALU.add)
        cqrep = small_p.tile([128, 512], BF16, tag="curep")
        nc.vector.tensor_copy(cqrep[:].rearrange("p (r q) -> p r q", r=8),
                              _bcast_mid(cq[:], 8))

        ALs = small_p.tile([BC, LS], F32, tag="ALs")
        g2a_full = pers_p.tile([128, 16 * 64], BF16)
        for c in range(2):
            sl = slice(c * 512, (c + 1) * 512)
            ps1 = ps_mm.tile([128, 512], F32, tag="mm")
            nc.tensor.matmul(ps1[0:64, :], lhsT=wsl("sA1top")[0:64, :],
                             rhs=YsT[0:64, sl], start=True, stop=False,
                             skip_group_check=True)
            nc.tensor.matmul(ps1[64:128, :], lhsT=wsl("sA1top")[64:128, :],
                             rhs=YsT[64:128, sl], start=True, stop=False,
                             skip_group_check=True)
            nc.tensor.matmul(ps1[:], lhsT=idb, rhs=cqrep[:],
                             start=False, stop=True, skip_group_check=True)
            g1 = chunk_p.tile([128, 512], BF16, tag="h1")
            nc.scalar.activation(g1[:], ps1[:], AF.Relu)
            ps2 = ps_mm.tile([128, 512], F32, tag="mm")
            nc.tensor.matmul(ps2[0:64, :], lhsT=wsl("sA2")[0:64, :],
                             rhs=g1[0:64, :], start=True, stop=False,
                             skip_group_check=True)
            nc.tensor.matmul(ps2[64:128, :], lhsT=wsl("sA2")[64:128, :],
                             rhs=g1[64:128, :], start=True, stop=False,
                             skip_group_check=True)
            nc.tensor.matmul(ps2[0:64, :], lhsT=rowv("b2s"),
                             rhs=ones[:, 0:512], start=False, stop=False,
                             skip_group_check=True)
            nc.tensor.matmul(ps2[64:128, :], lhsT=rowv("b2s"),
                             rhs=ones[:, 0:512], start=False, stop=True,
                             skip_group_check=True)
            g2a = g2a_full[:, sl]
            nc.vector.tensor_scalar(out=g2a, in0=ps2[:], scalar1=0.0,
                                    scalar2=fbc("a3s2"), op0=ALU.max,
                                    op1=ALU.mult)
            for h in range(2):
                psb = ps_bk.tile([64, 512], BF16, tag="bk")
                for j in range(4):
                    blk = h * 4 + j
                    nc.tensor.transpose(psb[0:64, j * 128:(j + 1) * 128],
                                        g2a[:, blk * 64:(blk + 1) * 64], idb)
                nc.vector.tensor_reduce(
                    out=ALs[:, c * 16 + h * 8:c * 16 + (h + 1) * 8],
                    in_=psb[:].rearrange("p (i f) -> p i f", f=64),
                    op=ALU.add, axis=AXL.X)

        nc.vector.tensor_tensor(out=ALs[:], in0=ALs[:], in1=smb[:],
                                op=ALU.add)
        nmxs = small_p.tile([BC, 1], F32, tag="nmxs")
        nc.vector.tensor_reduce(out=nmxs[:], in_=ALs[:], op=ALU.max,
                                axis=AXL.X, negate=True)
        exs = small_p.tile([BC, LS], BF16, tag="exs")
        nc.scalar.activation(exs[:], ALs[:], AF.Exp, bias=nmxs[:])
        sms = small_p.tile([BC, 1], F32, tag="sms")
        nc.vector.tensor_reduce(out=sms[:], in_=exs[:], op=ALU.add,
                                axis=AXL.X)
        recs = small_p.tile([BC, 1], F32, tag="recs")
        nc.vector.reciprocal(recs[:], sms[:])

        wtmps = pers_p.tile([BC, LS * D], BF16)
        nc.vector.tensor_tensor(
            out=wtmps[:].rearrange("p (i f) -> p i f", f=64),
            in0=Ys[:].rearrange("p (i f) -> p i f", f=64),
            in1=_bcast_inner(exs[:], 64), op=ALU.mult)
        hSr = small_p.tile([BC, D], F32, tag="hSr")
        nc.vector.tensor_reduce(
            out=hSr[:], in_=wtmps[:].rearrange("p (i f) -> p f i", f=64),
            op=ALU.add, axis=AXL.X)
        hS = small_p.tile([BC, D], BF16, tag="hS")
        nc.vector.tensor_scalar(out=hS[:], in0=hSr[:], scalar1=recs[:],
                                scalar2=None, op0=ALU.mult)

        # s_ln1
        psh = ps_sm.tile([128, 128], BF16, tag="psU")
        nc.tensor.transpose(psh[0:64, 0:64], hS[:], idb[0:64, 0:64])
        hST = small_p.tile([64, 64], BF16, tag="hST")
        nc.vector.tensor_copy(hST[:], psh[0:64, 0:64])
        pl1 = ps_sm.tile([128, 128], F32, tag="psc")
        nc.tensor.matmul(pl1[0:64, 0:64], lhsT=wsl("sWl1")[0:64, :],
                         rhs=hST[:], start=True, stop=False,
                         skip_group_check=True)
        nc.tensor.matmul(pl1[0:64, 0:64], lhsT=rowv("b1ls"),
                         rhs=ones[:, 0:64], start=False, stop=True,
                         skip_group_check=True)
        hS2T = small_p.tile([64, 64], BF16, tag="hS2T")
        nc.scalar.activation(hS2T[:], pl1[0:64, 0:64], AF.Relu)

        # s_ln2 on concat(hI_self, hS2)
        pf = ps_sm.tile([128, 128], F32, tag="psc")
        nc.tensor.matmul(pf[0:64, 0:64], lhsT=wsl("sWl2top")[0:64, :],
                         rhs=HIFT[0:64, 0:64], start=True, stop=False,
                         skip_group_check=True)
        nc.tensor.matmul(pf[0:64, 0:64], lhsT=wsl("sWl2bot")[0:64, :],
                         rhs=hS2T[:], start=False, stop=False,
                         skip_group_check=True)
        nc.tensor.matmul(pf[0:64, 0:64], lhsT=rowv("b2ls"),
                         rhs=ones[:, 0:64], start=False, stop=True,
                         skip_group_check=True)
        fT = small_p.tile([64, 64], BF16, tag="fT")
        nc.scalar.activation(fT[:], pf[0:64, 0:64], AF.Relu)

        # s_ln3
        po = ps_sm.tile([128, 128], F32, tag="psc")
        nc.tensor.matmul(po[0:64, 0:64], lhsT=wsl("sWl3")[0:64, :],
                         rhs=fT[:], start=True, stop=False,
                         skip_group_check=True)
        nc.tensor.matmul(po[0:64, 0:64], lhsT=rowv("b3ls"),
                         rhs=ones[:, 0:64], start=False, stop=True,
                         skip_group_check=True)
        oT = small_p.tile([64, 64], F32, tag="oT")
        nc.scalar.activation(oT[:], po[0:64, 0:64], AF.Relu)

        # transpose to row-major, write this core's shard
        pout = ps_sm.tile([128, 128], F32, tag="psc")
        nc.tensor.transpose(pout[0:64, 0:64], oT[:], idf[0:64, 0:64])
        orow = small_p.tile([64, 64], F32, tag="orow")
        nc.vector.tensor_copy(orow[:], pout[0:64, 0:64])
        nc.sync.dma_start(t_out[:], orow[:])

    nc.compile()
    return nc


def _prep(inputs):
    """Host-side constant folding + dtype prep (shared across cores)."""
    f32 = np.float32
    item_w = np.asarray(inputs["item_w"], f32)
    rating_w = np.asarray(inputs["rating_w"], f32)
    user_w = np.asarray(inputs["user_w"], f32)
    W1 = np.asarray(inputs["i_ln1_w"], f32)
    b1 = np.asarray(inputs["i_ln1_b"], f32)

    f8 = ml_dtypes.float8_e4m3fn
    tab = np.zeros((NTAB8, D), f8)
    tab[:NU] = (item_w @ W1[:64]).astype(f8)
    tab8 = tab.view(np.uint8)
    rtw1 = (rating_w @ W1[64:] + b1).astype(nbf)
    userb = user_w.astype(f8)
    ihist = np.asarray(inputs["item_hist"], np.int64)
    rhist = np.asarray(inputs["rating_hist"], np.int64)
    imask = np.asarray(inputs["item_mask"])

    def stack2(w):
        return np.vstack([w, w])

    wbcols = []
    i_att1 = np.asarray(inputs["i_att1_w"], f32)
    s_att1 = np.asarray(inputs["s_att1_w"], f32)
    s_ln2 = np.asarray(inputs["s_ln2_w"], f32)
    i_ln3 = np.asarray(inputs["i_ln3_w"], f32)
    for w in [i_att1[:64], i_att1[64:], np.asarray(inputs["i_att2_w"], f32),
              np.asarray(inputs["i_ln2_w"], f32), i_ln3[:64], i_ln3[64:],
              s_att1[:64], s_att1[64:], np.asarray(inputs["s_att2_w"], f32),
              np.asarray(inputs["s_ln1_w"], f32), s_ln2[:64], s_ln2[64:],
              np.asarray(inputs["s_ln3_w"], f32)]:
        wbcols.append(stack2(w))
    wbcols.append(np.eye(128, dtype=f32))
    wbmat = np.concatenate(wbcols, axis=1).astype(nbf)

    rowsv = np.concatenate(
        [np.asarray(inputs[k], f32) for k in
         ["i_att2_b", "i_ln2_b", "i_ln3_b", "s_att2_b", "s_ln1_b",
          "s_ln2_b", "s_ln3_b"]])[None, :].astype(nbf)

    fbm = np.zeros((128, 4), f32)
    fbm[:, 0] = np.tile(np.asarray(inputs["i_att1_b"], f32), 2)
    fbm[:, 1] = np.tile(np.asarray(inputs["i_att3_w"], f32)[:, 0], 2)
    fbm[:, 2] = np.tile(np.asarray(inputs["s_att1_b"], f32), 2)
    fbm[:, 3] = np.tile(np.asarray(inputs["s_att3_w"], f32)[:, 0], 2)

    # wb blob as rows-of-64 (p-major)
    wbrows = wbmat.reshape(128, 15, 64).reshape(1920, 64)

    def b128(a):
        v = np.ascontiguousarray(a).view(np.uint8).reshape(-1)
        assert v.size % 128 == 0, v.size
        return v.reshape(-1, 128)

    nodes = np.asarray(inputs["nodes"], np.int64)
    shist = np.asarray(inputs["social_hist"], np.int64)
    smask = np.asarray(inputs["social_mask"])
    in_maps = []
    for c in range(NCORES):
        nd = nodes[c * BC:(c + 1) * BC]
        nbrs = shist[nd]                           # [BC, LS]
        uids = np.zeros(NUID, np.int64)
        uids[:BC] = nd
        uids[BC:BC + BC * LS] = nbrs.reshape(-1)
        smb = np.where(smask[nd], 0.0, -1e9).astype(f32)
        aux = (2 * rhist[uids] + imask[uids]).astype(np.uint8)
        aux4 = (aux[:, 0::2] | (aux[:, 1::2] << 4)).astype(np.uint8)
        blob = np.concatenate([
            b128(wbrows),
            b128(userb[uids]),
            b128(ihist[uids].astype(np.uint16)),
            b128(aux4),
            b128(rowsv),
            b128(rtw1.reshape(1, 5 * D)),
            b128(smb),
            b128(fbm),
        ], axis=0)
        assert blob.shape[0] == RTOT, blob.shape
        in_maps.append({"tab": tab8, "big": blob.view(nbf).reshape(RTOT, D)})
    return in_maps


def _prep_sim0(inputs):
    """Single-core (core 0) input map for CoreSim."""
    return dict(_prep(inputs)[0])


def kernel(**inputs):
    if "nc" not in _CACHE:
        _CACHE["nc"] = build_program()
    nc = _CACHE["nc"]
    in_maps = _prep(inputs)
    res = run_bass_kernel_spmd(nc, in_maps, core_ids=list(range(NCORES)))
    return np.concatenate(
        [np.asarray(res.results[c]["out"]) for c in range(NCORES)],
        axis=0).astype(np.float32)


# revision 7
# speedup vs baseline: 140.2158x; 1.5293x over previous
"""Trainium2 Bass kernel for nn_Aggre_social (GNN message passing, social rec).

Strategy: data-parallel over the B=512 query users across 8 NeuronCores
(64 queries/core).  Per core we compute item_final() for 64 self uids +
64*32 = 2048 neighbor uids = 2112 uids (padded to 17 tiles of 128), each
with a 64-item history.

v3 layout (device-time optimized):
  - the folded item table itw1 = item_w @ i_ln1_w[:64] is staged fp8
    REPLICATED to every core, so there is no on-device AllGather;
  - item embedding rows are gathered on device with per-column indirect
    DMA ([128,1] indices -- the only indirect form this HW supports;
    batched multi-index offsets scramble and the dma_gather extended
    instruction is absent from the bedrock ucode image);
  - the 5-row rating table rtw1 = rating_w @ i_ln1_w[64:] + b1 is added
    via 5 select-multiply-add vector ops per tile (hides under the
    Pool-engine gather bottleneck);
  - per-(uid,item) item ids ship as uint16 rows, ratings and masks as
    packed 4-bit aux rows; user rows are host-gathered fp8;
  - each core writes only its own [64,64] output shard; the host
    concatenates (no output AllGather).
Compute is done mostly in bf16 with a feature-major ("transposed")
layout for the attention MLPs and a row-major layout for softmax /
weighted sums.
"""

import sys

sys.path.insert(0, "/opt/trn_rl_repo")

import numpy as np
import ml_dtypes
from contextlib import ExitStack

import concourse.bass as bass
import concourse.bacc as bacc
import concourse.tile as tile
from concourse import mybir
from concourse.bass import IndirectOffsetOnAxis, AP
from concourse.bass_utils import run_bass_kernel_spmd

BF16 = mybir.dt.bfloat16
F32 = mybir.dt.float32
I32 = mybir.dt.int32
U16 = mybir.dt.uint16
U8 = mybir.dt.uint8
F8E4 = mybir.dt.float8e4
AF = mybir.ActivationFunctionType
ALU = mybir.AluOpType
AXL = mybir.AxisListType

NCORES = 8
B = 512
BC = B // NCORES          # 64 queries per core
LS = 32
LI = 64
D = 64
NU = 50000
NTAB8 = 50048             # fp8 itw1 rows (64B each), padded
NT = 17                   # uid tiles per core (128 uids each)
NUID = NT * 128           # 2176 (2112 real + 64 pad)
NT2 = 18                  # padded to even for pair-blocking in phase 2
HIFROWS = NT2 * 128       # 2304 rows in HIF scratch

# blob region offsets, in 64-elem bf16 rows (128B each)
R_WB = 0                  # wb blob: 1920 rows
R_USER = 1920             # fp8 user rows: 2176 x 64B = 1088 rows
R_IHG = R_USER + 1088     # u16 item ids: 2176 rows
R_AUX = R_IHG + 2176      # 4-bit packed 2*rating+mask: 2176x32B = 544 rows
R_YS = R_AUX + 544        # i32 hif row index per (query,nbr) [64,32]: 64
R_ROWS = R_YS + 64        # bias row-vectors: 7 rows
R_RT = R_ROWS + 7         # folded rating rows: 5 rows
R_SMB = R_RT + 5          # social mask bias f32 [64,32]: 64 rows
R_FB = R_SMB + 64         # per-partition f32 bias/scale cols [128,4]: 16
RTOT = R_FB + 16          # 5884 rows = 753KB per core

nbf = ml_dtypes.bfloat16

_CACHE = {}


# ---------------------------------------------------------------------------
# run_bass_via_pjrt rebuilds its jit closure (and so retraces + relowers the
# whole program) on every call, which costs ~0.5s/dispatch of pure client-side
# work.  Memoize the jitted sharded callable per Bass program: semantics are
# identical (inputs are still concatenated, staged to the devices, executed
# and fetched on every call) -- only the jax trace/lower step is reused.
# run_bass_kernel_spmd remains the dispatch entry point.
# ---------------------------------------------------------------------------
_PJRT_JIT_CACHE = {}


def _install_pjrt_cache():
    from concourse import bass2jax as b2j
    import jax
    from jax.sharding import Mesh, PartitionSpec
    from jax.experimental.shard_map import shard_map

    orig = b2j.run_bass_via_pjrt
    if getattr(orig, "_aggre_cached", False):
        return

    def cached_run(nc, in_maps, n_cores):
        key = (id(nc), n_cores)
        if key not in _PJRT_JIT_CACHE:
            if nc.dbg_addr is not None or n_cores == 1:
                return orig(nc, in_maps, n_cores)  # uncommon paths: passthru
            b2j.install_neuronx_cc_hook()
            partition_name = (nc.partition_id_tensor.name
                              if nc.partition_id_tensor else None)
            in_names, out_names, out_avals, zero_shapes = [], [], [], []
            for alloc in nc.m.functions[0].allocations:
                if not isinstance(alloc, mybir.MemoryLocationSet):
                    continue
                name = alloc.memorylocations[0].name
                if alloc.kind == "ExternalInput":
                    if name != partition_name:
                        in_names.append(name)
                elif alloc.kind == "ExternalOutput":
                    out_names.append(name)
                    shape = tuple(alloc.tensor_shape)
                    dtype = mybir.dt.np(alloc.dtype)
                    out_avals.append(jax.core.ShapedArray(shape, dtype))
                    zero_shapes.append((shape, dtype))
            n_params = len(in_names)
            all_names = list(in_names) + out_names
            if partition_name is not None:
                all_names.append(partition_name)
            donate = tuple(range(n_params, n_params + len(out_names)))

            def _body(*args):
                operands = list(args)
                if partition_name is not None:
                    operands.append(b2j.partition_id_tensor())
                outs = b2j._bass_exec_p.bind(
                    *operands, out_avals=tuple(out_avals),
                    in_names=tuple(all_names), out_names=tuple(out_names),
                    lowering_input_output_aliases=(),
                    sim_require_finite=True, sim_require_nnan=True, nc=nc)
                return tuple(outs)

            mesh = Mesh(np.asarray(jax.devices()[:n_cores]), ("core",))
            sharded = jax.jit(
                shard_map(_body, mesh=mesh,
                          in_specs=(PartitionSpec("core"),)
                          * (n_params + len(out_names)),
                          out_specs=(PartitionSpec("core"),)
                          * len(out_names), check_rep=False),
                donate_argnums=donate, keep_unused=True)
            _PJRT_JIT_CACHE[key] = (nc, sharded, in_names, out_names,
                                    out_avals, zero_shapes)
        (_, sharded, in_names, out_names, out_avals,
         zero_shapes) = _PJRT_JIT_CACHE[key]
        n_params = len(in_names)
        concat_in = [
            np.concatenate([np.asarray(in_maps[c][name])
                            for c in range(n_cores)], axis=0)
            for name in in_names]
        # donate the previous call's device-resident output buffers (the
        # kernel overwrites every output element); zeros on the first call
        prev = _PJRT_JIT_CACHE.get(("prev_out", key))
        if prev is None:
            prev = [np.zeros((n_cores * s[0], *s[1:]), dt)
                    for s, dt in zero_shapes]
        out_arrs = sharded(*concat_in, *prev)
        _PJRT_JIT_CACHE[("prev_out", key)] = list(out_arrs)
        return [
            {name: np.asarray(out_arrs[i]).reshape(
                n_cores, *out_avals[i].shape)[c]
             for i, name in enumerate(out_names)}
            for c in range(n_cores)]

    cached_run._aggre_cached = True
    b2j.run_bass_via_pjrt = cached_run


_install_pjrt_cache()


def _bcast_mid(ap, rep):
    """[P, n] AP -> [P, rep, n] AP with stride-0 middle dim."""
    assert len(ap.ap) == 2
    return AP(ap.tensor, ap.offset, [ap.ap[0], [0, rep], ap.ap[1]])


def _bcast_inner(ap, rep):
    """[P, n] AP -> [P, n, rep] AP with stride-0 inner dim."""
    assert len(ap.ap) == 2
    return AP(ap.tensor, ap.offset, [ap.ap[0], ap.ap[1], [0, rep]])


def _stride2(ap, phase, n):
    """[P, 2n] AP -> [P, n] AP over elements phase, phase+2, ..."""
    assert len(ap.ap) == 2
    return AP(ap.tensor, ap.offset + phase, [ap.ap[0], [2, n]])


def build_program(Ks=(64,) * NT):
    assert len(Ks) == NT and all(8 <= k <= 64 and k % 8 == 0 for k in Ks)
    nc = bacc.Bacc("TRN2", target_bir_lowering=False, debug=False,
                   num_devices=NCORES)

    # ---- DRAM I/O ------------------------------------------------------
    def din(name, shape, dt):
        return nc.dram_tensor(name, shape, dt, kind="ExternalInput").ap()

    t_tab = din("tab", [NTAB8, D], U8)       # fp8 itw1 rows (64B each)
    t_big = din("big", [RTOT, D], BF16)      # everything else (see R_*)
    t_out = nc.dram_tensor("out", [BC, D], F32, kind="ExternalOutput").ap()

    W = {}  # weight blob column slots
    for i, name in enumerate(["A1top", "A1bot", "A2", "Wl2", "Wl3top",
                              "Wl3bot", "sA1top", "sA1bot", "sA2", "sWl1",
                              "sWl2top", "sWl2bot", "sWl3"]):
        W[name] = i * 64
    IDB_OFF = 13 * 64
    RW = {n: i * 64 for i, n in enumerate(
        ["b2a", "b2l", "b3l", "b2s", "b1ls", "b2ls", "b3ls"])}
    FB = {"b1a2": 0, "a3pk": 1, "b1s2": 2, "a3s2": 3}

    with tile.TileContext(nc) as tc, ExitStack() as ctx:
        const_p = ctx.enter_context(tc.tile_pool(name="const", bufs=1))
        small_p = ctx.enter_context(tc.tile_pool(name="small", bufs=3))
        big_p = ctx.enter_context(tc.tile_pool(name="big", bufs=2))
        chunk_p = ctx.enter_context(tc.tile_pool(name="chunk", bufs=4))
        pers_p = ctx.enter_context(tc.tile_pool(name="pers", bufs=1))
        dram_p = ctx.enter_context(tc.tile_pool(name="dram", bufs=1,
                                                space="DRAM"))
        ps_tr = ctx.enter_context(tc.tile_pool(name="ps_tr", bufs=2,
                                               space="PSUM"))
        ps_mm = ctx.enter_context(tc.tile_pool(name="ps_mm", bufs=2,
                                               space="PSUM"))
        ps_bk = ctx.enter_context(tc.tile_pool(name="ps_bk", bufs=2,
                                               space="PSUM"))
        ps_sm = ctx.enter_context(tc.tile_pool(name="ps_sm", bufs=1,
                                               space="PSUM"))

        # ---- constants into SBUF --------------------------------------
        # wb rows are p-major: partition p holds rows [15p, 15p+15)
        wb = const_p.tile([128, 13 * 64 + 128], BF16)
        nc.sync.dma_start(
            wb[:],
            t_big[R_WB:R_WB + 1920, :].rearrange("(p j) f -> p (j f)", p=128))
        rows = const_p.tile([1, 7 * 64], BF16)
        nc.sync.dma_start(rows[:], t_big[R_ROWS:R_ROWS + 7, :])
        fb = const_p.tile([128, 4], F32)
        nc.sync.dma_start(fb[:], t_big[R_FB:R_FB + 16, :].bitcast(F32))
        smb = const_p.tile([BC, LS], F32)
        nc.sync.dma_start(smb[:], t_big[R_SMB:R_SMB + 64, :].bitcast(F32))
        ones = const_p.tile([1, 512], BF16)
        nc.vector.memset(ones[:], 1.0)
        idb = wb[:, IDB_OFF:IDB_OFF + 128]      # bf16 identity
        idf = const_p.tile([128, 128], F32)     # f32 identity (0/1 exact)
        nc.vector.tensor_copy(idf[:], idb)

        # rating rows -> broadcast to all 128 partitions: rep5[p, k*64+f]
        rtrow = const_p.tile([1, 5 * D], BF16)
        nc.sync.dma_start(rtrow[:], t_big[R_RT:R_RT + 5, :])
        ps_r5 = ps_mm.tile([128, 5 * D], F32, tag="mm")
        nc.tensor.matmul(ps_r5[:], lhsT=ones[:, 0:128], rhs=rtrow[:],
                         start=True, stop=True)
        rep5 = const_p.tile([128, 5 * D], BF16)
        nc.vector.tensor_copy(rep5[:], ps_r5[:])

        def wsl(name):
            return wb[:, W[name]:W[name] + 64]

        def rowv(name):
            return rows[:, RW[name]:RW[name] + 64]

        def fbc(name):
            return fb[:, FB[name]:FB[name] + 1]

        HI_all = pers_p.tile([128, NT2 * 64], BF16)
        U_all = pers_p.tile([128, NT2 * 64], BF16)
        nc.vector.memset(HI_all[:, NT * 64:], 0.0)
        nc.vector.memset(U_all[:, NT * 64:], 0.0)
        hif = dram_p.tile([HIFROWS, D], BF16, tag="hif")
        assert hif[:].offset == 0

        # ================= PHASE 1: item_final per uid-tile =============
        # Host packs each uid's unmasked items first and sorts neighbor
        # slots by unmasked count, so tile t only processes Ks[t] columns.
        for t in range(NT):
            Kt = Ks[t]
            KD = Kt * D
            NCH = Kt // 8
            # user rows arrive fp8; widen to bf16 once
            UR8 = small_p.tile([128, D], U8, tag="UR8")
            nc.sync.dma_start(
                UR8[:],
                t_big[R_USER + t * 64:R_USER + (t + 1) * 64, :].bitcast(U8))
            UR = small_p.tile([128, D], BF16, tag="UR")
            nc.vector.tensor_copy(UR[:], UR8[:].bitcast(F8E4))
            nc.vector.tensor_copy(U_all[:, t * 64:(t + 1) * 64], UR[:])

            # --- gather x1 rows (fp8) from the table by item id ---------
            idx16 = small_p.tile([128, LI], U16, tag="idx16")
            nc.sync.dma_start(
                idx16[:],
                t_big[R_IHG + t * 128:R_IHG + (t + 1) * 128, :].bitcast(U16))
            idx32 = small_p.tile([128, LI], I32, tag="idx32")
            nc.vector.tensor_copy(idx32[:], idx16[:])
            x1f8 = big_p.tile([128, LI * D], U8, tag="x1f8")
            for i in range(Kt):
                nc.gpsimd.indirect_dma_start(
                    out=x1f8[:, i * D:(i + 1) * D], out_offset=None,
                    in_=t_tab,
                    in_offset=IndirectOffsetOnAxis(ap=idx32[:, i:i + 1],
                                                   axis=0))
            x1 = big_p.tile([128, LI * D], BF16, tag="x1")
            nc.vector.tensor_copy(x1[:, :KD],
                                  x1f8[:, :KD].bitcast(F8E4))

            # --- unpack aux nibbles: item 2j in low, 2j+1 in high -------
            aux4 = small_p.tile([128, LI // 2], U8, tag="aux4")
            nc.sync.dma_start(
                aux4[:],
                t_big[R_AUX + t * 32:R_AUX + (t + 1) * 32, :].bitcast(U8))
            aux8 = small_p.tile([128, LI], U8, tag="aux8")
            nc.vector.tensor_scalar(out=_stride2(aux8[:], 0, LI // 2),
                                    in0=aux4[:], scalar1=15, scalar2=None,
                                    op0=ALU.bitwise_and)
            nc.vector.tensor_scalar(out=_stride2(aux8[:], 1, LI // 2),
                                    in0=aux4[:], scalar1=4, scalar2=None,
                                    op0=ALU.logical_shift_right)
            r8 = small_p.tile([128, LI], U8, tag="r8")
            nc.vector.tensor_scalar(out=r8[:], in0=aux8[:], scalar1=1,
                                    scalar2=None,
                                    op0=ALU.logical_shift_right)
            rrf = small_p.tile([128, LI], BF16, tag="rrf")
            nc.vector.tensor_copy(rrf[:, :Kt], r8[:, :Kt])
            rtmp = big_p.tile([128, LI * D], BF16, tag="rtmp")
            for k in range(5):
                mk = small_p.tile([128, LI], BF16, tag="mk")
                nc.vector.tensor_scalar(out=mk[:, :Kt], in0=rrf[:, :Kt],
                                        scalar1=float(k), scalar2=None,
                                        op0=ALU.is_equal)
                nc.vector.tensor_tensor(
                    out=rtmp[:, :KD].rearrange("p (i f) -> p i f", f=64),
                    in0=_bcast_inner(mk[:, :Kt], 64),
                    in1=_bcast_mid(rep5[:, k * D:(k + 1) * D], Kt),
                    op=ALU.mult)
                nc.vector.tensor_tensor(out=x1[:, :KD], in0=x1[:, :KD],
                                        in1=rtmp[:, :KD], op=ALU.add)

            # --- mask bias rows: IMB = (mask - 1) * 1e9 -----------------
            mm8 = small_p.tile([128, LI], U8, tag="mm8")
            nc.vector.tensor_scalar(out=mm8[:], in0=aux8[:], scalar1=1,
                                    scalar2=None, op0=ALU.bitwise_and)
            mmf = small_p.tile([128, LI], F32, tag="mmf")
            nc.vector.tensor_copy(mmf[:], mm8[:])
            IMB = small_p.tile([128, LI], F32, tag="IMB")
            nc.vector.tensor_scalar(out=IMB[:], in0=mmf[:], scalar1=1.0,
                                    scalar2=1e9, op0=ALU.subtract,
                                    op1=ALU.mult)

            # x = relu(x1)   (row-major canonical x)
            xR = big_p.tile([128, LI * D], BF16, tag="xR")
            nc.scalar.activation(xR[:, :KD], x1[:, :KD], AF.Relu)

            # forward transposes -> feature-major packed pairs
            xT = big_p.tile([128, LI * D], BF16, tag="xT")
            for g in range(NCH):     # groups of 4 item-pair blocks
                pst = ps_tr.tile([128, 512], BF16, tag="tr")
                for j in range(4):
                    blk = g * 4 + j
                    nc.tensor.transpose(
                        pst[:, j * 128:(j + 1) * 128],
                        xR[:, blk * 128:(blk + 1) * 128], idb)
                eng = nc.scalar if g % 2 == 0 else nc.vector
                if eng is nc.scalar:
                    nc.scalar.copy(xT[:, g * 512:(g + 1) * 512], pst[:])
                else:
                    nc.vector.tensor_copy(xT[:, g * 512:(g + 1) * 512],
                                          pst[:])

            # c_u = A1bot^T u + b1  (both halves)
            psU = ps_sm.tile([128, 128], BF16, tag="psU")
            nc.tensor.transpose(psU[0:64, :], UR[:, 0:64], idb)
            nc.tensor.transpose(psU[64:128, :], UR[:, 0:64], idb)
            UT2 = small_p.tile([128, 128], BF16, tag="UT2")
            nc.vector.tensor_copy(UT2[:], psU[:])
            psc = ps_sm.tile([128, 128], F32, tag="psc")
            nc.tensor.matmul(psc[0:64, :], lhsT=wsl("A1bot")[0:64, :],
                             rhs=UT2[0:64, :], start=True, stop=True)
            nc.tensor.matmul(psc[64:128, :], lhsT=wsl("A1bot")[64:128, :],
                             rhs=UT2[64:128, :], start=True, stop=True)
            cu = small_p.tile([128, 128], BF16, tag="cu")
            nc.vector.tensor_scalar(out=cu[:], in0=psc[:],
                                    scalar1=fbc("b1a2"), scalar2=None,
                                    op0=ALU.add)
            curep = small_p.tile([128, 512], BF16, tag="curep")
            nc.vector.tensor_copy(curep[:].rearrange("p (r q) -> p r q", r=4),
                                  _bcast_mid(cu[:], 4))

            AL = small_p.tile([128, LI], F32, tag="AL")
            h2a_full = big_p.tile([128, LI * D], BF16, tag="h2a")
            for c in range(NCH):
                sl = slice(c * 512, (c + 1) * 512)
                # --- att layer 1 (x part + u part via identity-matmul)
                ps1 = ps_mm.tile([128, 512], F32, tag="mm")
                nc.tensor.matmul(ps1[0:64, :], lhsT=wsl("A1top")[0:64, :],
                                 rhs=xT[0:64, sl], start=True, stop=False,
                                 skip_group_check=True)
                nc.tensor.matmul(ps1[64:128, :], lhsT=wsl("A1top")[64:128, :],
                                 rhs=xT[64:128, sl], start=True, stop=False,
                                 skip_group_check=True)
                nc.tensor.matmul(ps1[:], lhsT=idb, rhs=curep[:],
                                 start=False, stop=True,
                                 skip_group_check=True)
                h1 = chunk_p.tile([128, 512], BF16, tag="h1")
                nc.scalar.activation(h1[:], ps1[:], AF.Relu)
                # --- att layer 2 + bias row + (relu, * a3) on DVE
                ps2 = ps_mm.tile([128, 512], F32, tag="mm")
                nc.tensor.matmul(ps2[0:64, :], lhsT=wsl("A2")[0:64, :],
                                 rhs=h1[0:64, :], start=True, stop=False,
                                 skip_group_check=True)
                nc.tensor.matmul(ps2[64:128, :], lhsT=wsl("A2")[64:128, :],
                                 rhs=h1[64:128, :], start=True, stop=False,
                                 skip_group_check=True)
                nc.tensor.matmul(ps2[0:64, :], lhsT=rowv("b2a"),
                                 rhs=ones[:, 0:512], start=False, stop=False,
                                 skip_group_check=True)
                nc.tensor.matmul(ps2[64:128, :], lhsT=rowv("b2a"),
                                 rhs=ones[:, 0:512], start=False, stop=True,
                                 skip_group_check=True)
                h2a = h2a_full[:, sl]
                nc.vector.tensor_scalar(out=h2a, in0=ps2[:], scalar1=0.0,
                                        scalar2=fbc("a3pk"), op0=ALU.max,
                                        op1=ALU.mult)
                # --- att layer 3: back-transpose + grouped partition sum
                psb = ps_bk.tile([128, 512], BF16, tag="bk")
                for j in range(4):
                    nc.tensor.transpose(psb[:, j * 128:(j + 1) * 128],
                                        h2a[:, j * 128:(j + 1) * 128], idb)
                nc.vector.tensor_reduce(
                    out=AL[:, c * 8:(c + 1) * 8],
                    in_=psb[:].rearrange("p (i f) -> p i f", f=64),
                    op=ALU.add, axis=AXL.X)

            # --- masked softmax over items
            nc.vector.tensor_tensor(out=AL[:, :Kt], in0=AL[:, :Kt],
                                    in1=IMB[:, :Kt], op=ALU.add)
            nmx = small_p.tile([128, 1], F32, tag="nmx")
            nc.vector.tensor_reduce(out=nmx[:], in_=AL[:, :Kt], op=ALU.max,
                                    axis=AXL.X, negate=True)
            ex = small_p.tile([128, LI], BF16, tag="ex")
            nc.scalar.activation(ex[:, :Kt], AL[:, :Kt], AF.Exp, bias=nmx[:])
            sm = small_p.tile([128, 1], F32, tag="sm")
            nc.vector.tensor_reduce(out=sm[:], in_=ex[:, :Kt], op=ALU.add,
                                    axis=AXL.X)
            rec = small_p.tile([128, 1], F32, tag="rec")
            nc.vector.reciprocal(rec[:], sm[:])

            # --- weighted sum over items (row-major)
            wtmp = big_p.tile([128, LI * D], BF16, tag="wtmp")
            nc.vector.tensor_tensor(
                out=wtmp[:, :KD].rearrange("p (i f) -> p i f", f=64),
                in0=xR[:, :KD].rearrange("p (i f) -> p i f", f=64),
                in1=_bcast_inner(ex[:, :Kt], 64), op=ALU.mult)
            hIr = small_p.tile([128, D], F32, tag="hIr")
            nc.vector.tensor_reduce(
                out=hIr[:],
                in_=wtmp[:, :KD].rearrange("p (i f) -> p f i", f=64),
                op=ALU.add, axis=AXL.X)
            nc.vector.tensor_scalar(out=HI_all[:, t * 64:(t + 1) * 64],
                                    in0=hIr[:], scalar1=rec[:], scalar2=None,
                                    op0=ALU.mult)

        # ================= PHASE 2: i_ln2 / i_ln3 for all uids ==========
        hIT = pers_p.tile([128, NT2 * 64], BF16)
        uT = pers_p.tile([128, NT2 * 64], BF16)
        for b in range(NT2 // 2):
            pst = ps_tr.tile([128, 512], BF16, tag="tr")
            nc.tensor.transpose(pst[:, 0:128],
                                HI_all[:, b * 128:(b + 1) * 128], idb)
            nc.tensor.transpose(pst[:, 128:256],
                                U_all[:, b * 128:(b + 1) * 128], idb)
            nc.scalar.copy(hIT[:, b * 128:(b + 1) * 128], pst[:, 0:128])
            nc.scalar.copy(uT[:, b * 128:(b + 1) * 128], pst[:, 128:256])

        hI2T = pers_p.tile([128, NT2 * 64], BF16)
        HIFT = pers_p.tile([128, NT2 * 64], BF16)
        csizes = [512, 512, 128]
        off = 0
        for cs in csizes:
            sl = slice(off, off + cs)
            psl = ps_mm.tile([128, 512], F32, tag="mm")
            nc.tensor.matmul(psl[0:64, 0:cs], lhsT=wsl("Wl2")[0:64, :],
                             rhs=hIT[0:64, sl], start=True, stop=False,
                             skip_group_check=True)
            nc.tensor.matmul(psl[64:128, 0:cs], lhsT=wsl("Wl2")[64:128, :],
                             rhs=hIT[64:128, sl], start=True, stop=False,
                             skip_group_check=True)
            nc.tensor.matmul(psl[0:64, 0:cs], lhsT=rowv("b2l"),
                             rhs=ones[:, 0:cs], start=False, stop=False,
                             skip_group_check=True)
            nc.tensor.matmul(psl[64:128, 0:cs], lhsT=rowv("b2l"),
                             rhs=ones[:, 0:cs], start=False, stop=True,
                             skip_group_check=True)
            nc.scalar.activation(hI2T[:, sl], psl[:, 0:cs], AF.Relu)

            psf = ps_mm.tile([128, 512], F32, tag="mm")
            nc.tensor.matmul(psf[0:64, 0:cs], lhsT=wsl("Wl3bot")[0:64, :],
                             rhs=hI2T[0:64, sl], start=True, stop=False,
                             skip_group_check=True)
            nc.tensor.matmul(psf[64:128, 0:cs], lhsT=wsl("Wl3bot")[64:128, :],
                             rhs=hI2T[64:128, sl], start=True, stop=False,
                             skip_group_check=True)
            nc.tensor.matmul(psf[0:64, 0:cs], lhsT=wsl("Wl3top")[0:64, :],
                             rhs=uT[0:64, sl], start=False, stop=False,
                             skip_group_check=True)
            nc.tensor.matmul(psf[64:128, 0:cs], lhsT=wsl("Wl3top")[64:128, :],
                             rhs=uT[64:128, sl], start=False, stop=False,
                             skip_group_check=True)
            nc.tensor.matmul(psf[0:64, 0:cs], lhsT=rowv("b3l"),
                             rhs=ones[:, 0:cs], start=False, stop=False,
                             skip_group_check=True)
            nc.tensor.matmul(psf[64:128, 0:cs], lhsT=rowv("b3l"),
                             rhs=ones[:, 0:cs], start=False, stop=True,
                             skip_group_check=True)
            nc.scalar.activation(HIFT[:, sl], psf[:, 0:cs], AF.Relu)
            off += cs

        # write HIF rows to DRAM scratch (row-major)
        for b in range(NT2 // 2):
            psw = ps_tr.tile([128, 512], BF16, tag="tr")
            nc.tensor.transpose(psw[:, 0:128],
                                HIFT[:, b * 128:(b + 1) * 128], idb)
            hrow = small_p.tile([128, 128], BF16, tag="hrow")
            nc.vector.tensor_copy(hrow[:], psw[:, 0:128])
            nc.sync.dma_start(hif[(2 * b) * 128:(2 * b + 1) * 128, :],
                              hrow[:, 0:64])
            nc.sync.dma_start(hif[(2 * b + 1) * 128:(2 * b + 2) * 128, :],
                              hrow[:, 64:128])

        # ================= PHASE 3: social aggregation ==================
        # neighbor slots are k-sorted, so gather each query's neighbor
        # rows from hif by the host-provided new-slot index
        ysx = small_p.tile([BC, LS], I32, tag="ysx")
        nc.sync.dma_start(ysx[:], t_big[R_YS:R_YS + 64, :].bitcast(I32))
        Ys = pers_p.tile([BC, LS * D], BF16)
        for j in range(LS):
            nc.gpsimd.indirect_dma_start(
                out=Ys[:, j * D:(j + 1) * D], out_offset=None,
                in_=hif[:],
                in_offset=IndirectOffsetOnAxis(ap=ysx[:, j:j + 1], axis=0))

        YsT = pers_p.tile([128, 16 * 64], BF16)
        for g in range(4):
            pst = ps_tr.tile([128, 512], BF16, tag="tr")
            for j in range(4):
                blk = g * 4 + j
                nc.tensor.transpose(pst[0:128, j * 64:(j + 1) * 64],
                                    Ys[:, blk * 128:(blk + 1) * 128],
                                    idb[0:64, 0:64])
            nc.scalar.copy(YsT[:, g * 256:(g + 1) * 256], pst[:, 0:256])

        psU = ps_sm.tile([128, 128], BF16, tag="psU")
        nc.tensor.transpose(psU[0:64, 0:64], U_all[0:64, 0:64],
                            idb[0:64, 0:64])
        nc.tensor.transpose(psU[64:128, 0:64], U_all[0:64, 0:64],
                            idb[0:64, 0:64])
        UqT = small_p.tile([128, 64], BF16, tag="UqT")
        nc.vector.tensor_copy(UqT[:], psU[:, 0:64])
        pscq = ps_sm.tile([128, 128], F32, tag="psc")
        nc.tensor.matmul(pscq[0:64, 0:64], lhsT=wsl("sA1bot")[0:64, :],
                         rhs=UqT[0:64, :], start=True, stop=True)
        nc.tensor.matmul(pscq[64:128, 0:64], lhsT=wsl("sA1bot")[64:128, :],
                         rhs=UqT[64:128, :], start=True, stop=True)
        cq = small_p.tile([128, 64], BF16, tag="cq")
        nc.vector.tensor_scalar(out=cq[:], in0=pscq[:, 0:64],
                                scalar1=fbc("b1s2"), scalar2=None,
                                op0=ALU.add)
        cqrep = small_p.tile([128, 512], BF16, tag="curep")
        nc.vector.tensor_copy(cqrep[:].rearrange("p (r q) -> p r q", r=8),
                              _bcast_mid(cq[:], 8))

        ALs = small_p.tile([BC, LS], F32, tag="ALs")
        g2a_full = pers_p.tile([128, 16 * 64], BF16)
        for c in range(2):
            sl = slice(c * 512, (c + 1) * 512)
            ps1 = ps_mm.tile([128, 512], F32, tag="mm")
            nc.tensor.matmul(ps1[0:64, :], lhsT=wsl("sA1top")[0:64, :],
                             rhs=YsT[0:64, sl], start=True, stop=False,
                             skip_group_check=True)
            nc.tensor.matmul(ps1[64:128, :], lhsT=wsl("sA1top")[64:128, :],
                             rhs=YsT[64:128, sl], start=True, stop=False,
                             skip_group_check=True)
            nc.tensor.matmul(ps1[:], lhsT=idb, rhs=cqrep[:],
                             start=False, stop=True, skip_group_check=True)
            g1 = chunk_p.tile([128, 512], BF16, tag="h1")
            nc.scalar.activation(g1[:], ps1[:], AF.Relu)
            ps2 = ps_mm.tile([128, 512], F32, tag="mm")
            nc.tensor.matmul(ps2[0:64, :], lhsT=wsl("sA2")[0:64, :],
                             rhs=g1[0:64, :], start=True, stop=False,
                             skip_group_check=True)
            nc.tensor.matmul(ps2[64:128, :], lhsT=wsl("sA2")[64:128, :],
                             rhs=g1[64:128, :], start=True, stop=False,
                             skip_group_check=True)
            nc.tensor.matmul(ps2[0:64, :], lhsT=rowv("b2s"),
                             rhs=ones[:, 0:512], start=False, stop=False,
                             skip_group_check=True)
            nc.tensor.matmul(ps2[64:128, :], lhsT=rowv("b2s"),
                             rhs=ones[:, 0:512], start=False, stop=True,
                             skip_group_check=True)
            g2a = g2a_full[:, sl]
            nc.vector.tensor_scalar(out=g2a, in0=ps2[:], scalar1=0.0,
                                    scalar2=fbc("a3s2"), op0=ALU.max,
                                    op1=ALU.mult)
            for h in range(2):
                psb = ps_bk.tile([64, 512], BF16, tag="bk")
                for j in range(4):
                    blk = h * 4 + j
                    nc.tensor.transpose(psb[0:64, j * 128:(j + 1) * 128],
                                        g2a[:, blk * 64:(blk + 1) * 64], idb)
                nc.vector.tensor_reduce(
                    out=ALs[:, c * 16 + h * 8:c * 16 + (h + 1) * 8],
                    in_=psb[:].rearrange("p (i f) -> p i f", f=64),
                    op=ALU.add, axis=AXL.X)

        nc.vector.tensor_tensor(out=ALs[:], in0=ALs[:], in1=smb[:],
                                op=ALU.add)
        nmxs = small_p.tile([BC, 1], F32, tag="nmxs")
        nc.vector.tensor_reduce(out=nmxs[:], in_=ALs[:], op=ALU.max,
                                axis=AXL.X, negate=True)
        exs = small_p.tile([BC, LS], BF16, tag="exs")
        nc.scalar.activation(exs[:], ALs[:], AF.Exp, bias=nmxs[:])
        sms = small_p.tile([BC, 1], F32, tag="sms")
        nc.vector.tensor_reduce(out=sms[:], in_=exs[:], op=ALU.add,
                                axis=AXL.X)
        recs = small_p.tile([BC, 1], F32, tag="recs")
        nc.vector.reciprocal(recs[:], sms[:])

        wtmps = pers_p.tile([BC, LS * D], BF16)
        nc.vector.tensor_tensor(
            out=wtmps[:].rearrange("p (i f) -> p i f", f=64),
            in0=Ys[:].rearrange("p (i f) -> p i f", f=64),
            in1=_bcast_inner(exs[:], 64), op=ALU.mult)
        hSr = small_p.tile([BC, D], F32, tag="hSr")
        nc.vector.tensor_reduce(
            out=hSr[:], in_=wtmps[:].rearrange("p (i f) -> p f i", f=64),
            op=ALU.add, axis=AXL.X)
        hS = small_p.tile([BC, D], BF16, tag="hS")
        nc.vector.tensor_scalar(out=hS[:], in0=hSr[:], scalar1=recs[:],
                                scalar2=None, op0=ALU.mult)

        # s_ln1
        psh = ps_sm.tile([128, 128], BF16, tag="psU")
        nc.tensor.transpose(psh[0:64, 0:64], hS[:], idb[0:64, 0:64])
        hST = small_p.tile([64, 64], BF16, tag="hST")
        nc.vector.tensor_copy(hST[:], psh[0:64, 0:64])
        pl1 = ps_sm.tile([128, 128], F32, tag="psc")
        nc.tensor.matmul(pl1[0:64, 0:64], lhsT=wsl("sWl1")[0:64, :],
                         rhs=hST[:], start=True, stop=False,
                         skip_group_check=True)
        nc.tensor.matmul(pl1[0:64, 0:64], lhsT=rowv("b1ls"),
                         rhs=ones[:, 0:64], start=False, stop=True,
                         skip_group_check=True)
        hS2T = small_p.tile([64, 64], BF16, tag="hS2T")
        nc.scalar.activation(hS2T[:], pl1[0:64, 0:64], AF.Relu)

        # s_ln2 on concat(hI_self, hS2)
        pf = ps_sm.tile([128, 128], F32, tag="psc")
        nc.tensor.matmul(pf[0:64, 0:64], lhsT=wsl("sWl2top")[0:64, :],
                         rhs=HIFT[0:64, 0:64], start=True, stop=False,
                         skip_group_check=True)
        nc.tensor.matmul(pf[0:64, 0:64], lhsT=wsl("sWl2bot")[0:64, :],
                         rhs=hS2T[:], start=False, stop=False,
                         skip_group_check=True)
        nc.tensor.matmul(pf[0:64, 0:64], lhsT=rowv("b2ls"),
                         rhs=ones[:, 0:64], start=False, stop=True,
                         skip_group_check=True)
        fT = small_p.tile([64, 64], BF16, tag="fT")
        nc.scalar.activation(fT[:], pf[0:64, 0:64], AF.Relu)

        # s_ln3
        po = ps_sm.tile([128, 128], F32, tag="psc")
        nc.tensor.matmul(po[0:64, 0:64], lhsT=wsl("sWl3")[0:64, :],
                         rhs=fT[:], start=True, stop=False,
                         skip_group_check=True)
        nc.tensor.matmul(po[0:64, 0:64], lhsT=rowv("b3ls"),
                         rhs=ones[:, 0:64], start=False, stop=True,
                         skip_group_check=True)
        oT = small_p.tile([64, 64], F32, tag="oT")
        nc.scalar.activation(oT[:], po[0:64, 0:64], AF.Relu)

        # transpose to row-major, write this core's shard
        pout = ps_sm.tile([128, 128], F32, tag="psc")
        nc.tensor.transpose(pout[0:64, 0:64], oT[:], idf[0:64, 0:64])
        orow = small_p.tile([64, 64], F32, tag="orow")
        nc.vector.tensor_copy(orow[:], pout[0:64, 0:64])
        nc.sync.dma_start(t_out[:], orow[:])

    nc.compile()
    return nc


def _prep(inputs):
    """Host-side constant folding + dtype prep (shared across cores)."""
    f32 = np.float32
    item_w = np.asarray(inputs["item_w"], f32)
    rating_w = np.asarray(inputs["rating_w"], f32)
    user_w = np.asarray(inputs["user_w"], f32)
    W1 = np.asarray(inputs["i_ln1_w"], f32)
    b1 = np.asarray(inputs["i_ln1_b"], f32)

    f8 = ml_dtypes.float8_e4m3fn
    tab = np.zeros((NTAB8, D), f8)
    tab[:NU] = (item_w @ W1[:64]).astype(f8)
    tab8 = tab.view(np.uint8)
    rtw1 = (rating_w @ W1[64:] + b1).astype(nbf)
    userb = user_w.astype(f8)
    ihist = np.asarray(inputs["item_hist"], np.int64)
    rhist = np.asarray(inputs["rating_hist"], np.int64)
    imask = np.asarray(inputs["item_mask"])

    def stack2(w):
        return np.vstack([w, w])

    wbcols = []
    i_att1 = np.asarray(inputs["i_att1_w"], f32)
    s_att1 = np.asarray(inputs["s_att1_w"], f32)
    s_ln2 = np.asarray(inputs["s_ln2_w"], f32)
    i_ln3 = np.asarray(inputs["i_ln3_w"], f32)
    for w in [i_att1[:64], i_att1[64:], np.asarray(inputs["i_att2_w"], f32),
              np.asarray(inputs["i_ln2_w"], f32), i_ln3[:64], i_ln3[64:],
              s_att1[:64], s_att1[64:], np.asarray(inputs["s_att2_w"], f32),
              np.asarray(inputs["s_ln1_w"], f32), s_ln2[:64], s_ln2[64:],
              np.asarray(inputs["s_ln3_w"], f32)]:
        wbcols.append(stack2(w))
    wbcols.append(np.eye(128, dtype=f32))
    wbmat = np.concatenate(wbcols, axis=1).astype(nbf)

    rowsv = np.concatenate(
        [np.asarray(inputs[k], f32) for k in
         ["i_att2_b", "i_ln2_b", "i_ln3_b", "s_att2_b", "s_ln1_b",
          "s_ln2_b", "s_ln3_b"]])[None, :].astype(nbf)

    fbm = np.zeros((128, 4), f32)
    fbm[:, 0] = np.tile(np.asarray(inputs["i_att1_b"], f32), 2)
    fbm[:, 1] = np.tile(np.asarray(inputs["i_att3_w"], f32)[:, 0], 2)
    fbm[:, 2] = np.tile(np.asarray(inputs["s_att1_b"], f32), 2)
    fbm[:, 3] = np.tile(np.asarray(inputs["s_att3_w"], f32)[:, 0], 2)

    # wb blob as rows-of-64 (p-major)
    wbrows = wbmat.reshape(128, 15, 64).reshape(1920, 64)

    def b128(a):
        v = np.ascontiguousarray(a).view(np.uint8).reshape(-1)
        assert v.size % 128 == 0, v.size
        return v.reshape(-1, 128)

    nodes = np.asarray(inputs["nodes"], np.int64)
    shist = np.asarray(inputs["social_hist"], np.int64)
    smask = np.asarray(inputs["social_mask"])
    in_maps = []
    ks_all = np.zeros((NCORES, NT), np.int64)
    for c in range(NCORES):
        nd = nodes[c * BC:(c + 1) * BC]
        nbrs = shist[nd]                           # [BC, LS]
        uids = np.zeros(NUID, np.int64)
        uids[:BC] = nd
        uids[BC:BC + BC * LS] = nbrs.reshape(-1)
        smb = np.where(smask[nd], 0.0, -1e9).astype(f32)
        # pack each uid's unmasked items first (attention over the item
        # set is permutation-invariant), so tiles only need to process up
        # to the tile-max unmasked count
        ih, rh = ihist[uids], rhist[uids]
        im = imask[uids].astype(bool)
        perm = np.argsort(~im, axis=1, kind="stable")
        ih = np.take_along_axis(ih, perm, 1)
        rh = np.take_along_axis(rh, perm, 1)
        im = np.take_along_axis(im, perm, 1)
        k = im.sum(1).astype(np.int64)
        k[BC + BC * LS:] = 0                       # pad slots sort last
        # sort neighbor slots by k desc; self slots stay at 0..BC-1
        order = np.argsort(-k[BC:], kind="stable")
        slotmap = np.concatenate([np.arange(BC), BC + order])
        ih, rh, im = ih[slotmap], rh[slotmap], im[slotmap]
        uods, ks = uids[slotmap], k[slotmap]
        inv = np.zeros(NUID, np.int64)
        inv[slotmap] = np.arange(NUID)
        ysidx = inv[BC + np.arange(BC * LS)].reshape(BC, LS).astype(np.int32)
        for t in range(NT):
            ks_all[c, t] = max(8, -(-int(ks[t * 128:(t + 1) * 128].max())
                                    // 8) * 8)
        aux = (2 * rh + im).astype(np.uint8)
        aux4 = (aux[:, 0::2] | (aux[:, 1::2] << 4)).astype(np.uint8)
        blob = np.concatenate([
            b128(wbrows),
            b128(userb[uods]),
            b128(ih.astype(np.uint16)),
            b128(aux4),
            b128(ysidx),
            b128(rowsv),
            b128(rtw1.reshape(1, 5 * D)),
            b128(smb),
            b128(fbm),
        ], axis=0)
        assert blob.shape[0] == RTOT, blob.shape
        in_maps.append({"tab": tab8, "big": blob.view(nbf).reshape(RTOT, D)})
    _CACHE["Ks"] = tuple(int(x) for x in ks_all.max(0))
    return in_maps


def _prep_sim0(inputs):
    """Single-core (core 0) input map for CoreSim."""
    return dict(_prep(inputs)[0])


def kernel(**inputs):
    in_maps = _prep(inputs)
    key = ("nc", _CACHE["Ks"])
    if key not in _CACHE:
        _CACHE[key] = build_program(_CACHE["Ks"])
    nc = _CACHE["nc"] = _CACHE[key]
    res = run_bass_kernel_spmd(nc, in_maps, core_ids=list(range(NCORES)))
    return np.concatenate(
        [np.asarray(res.results[c]["out"]) for c in range(NCORES)],
        axis=0).astype(np.float32)
